# revision 6
# baseline (speedup 1.0000x reference)
"""Minibatch discrimination kernel for 8 Trainium2 NeuronCores.

Reference computation:
    m = (x @ T.reshape(512, 128*32)).reshape(B=128, O=128, K=32)
    norm[i,j,o] = sum_k |m[i,o,k] - m[j,o,k]|
    o_b[j,o]    = sum_i exp(-norm[i,j,o]) - 1
    out         = concat([x, o_b], axis=1)            # [128, 640]

Distribution: shard the output-feature dim O=128 across the 8 cores (16
o's per core); no collectives.  Each core runs the GEMM for its T-slice
and the BxB pairwise exp-sum for its o-slice.

Per-core dataflow (tiles are [partition, free]):
  - GEMM -> M per o-group g as [(4o x 32k)=128 partitions, i=128] in
    PSUM; evicted to bf16 m_bf plus an exact f32 upcast m32 (the
    per-partition scalar / activation bias source).
  - norm is symmetric, so quad q (4 j's) only computes columns
    i >= 4q: free dim shrinks 128 -> 4 across quads, halving the
    elementwise volume.  The missing i < 4q part of o_b comes back via
    per-quad column sums (see below).
  - |d| tiles, one fused op per (j-region, o-group):
      DVE / GpSimd: tensor_scalar(subtract, max, 0) = relu(m_i - m_j)
        (weight-2 selector + P-correction; abs is not in the DVE/Pool
        hw ISA),
      ScalarE: activation Abs(-m + bias m32[:,j]) = |d| directly
        (weight-1 selector, no correction).
    A static plan balances the three engines' busy time.
  - k-reduction on the TensorEngine: per quad one seed matmul deposits
    -P_S[i,o] (P_S = sum over the RELU-produced groups of that row's
    region, host-precomputed per quad in c1) and per tile a selector
    matmul accumulates into the [(4j x 32(16o+16pad)), i] PSUM tile.
    Region jj=0 packs its tiles as fp8 pairs consumed by DoubleRow
    matmuls (0.5 cyc/row; hw requires dst partition base 0, so only
    this region qualifies).
  - One ScalarE Exp per quad with bias +P_S[j,o] (rides the bq table)
    -> E_q in SBUF bf16.  Row sums via DVE free-axis reduce into
    rs[:, q]; column sums via one PE matmul per quad accumulating
    sel4^T @ E_q[:, 4:] into ACC[16 o, j] over columns j >= 4q+4 only,
    so ACC[o,j] ends as sum_{i<4q_j} E[i,j] (prefix by construction).
  - Host combines o_b[j,o] = rs + ACC - 1 and concats with x.
"""

import numpy as np
import ml_dtypes

import concourse.bacc as bacc
import concourse.tile as tile
import concourse.mybir as mybir
from concourse.bass_utils import run_bass_kernel_spmd

BF16 = ml_dtypes.bfloat16
FP8 = ml_dtypes.float8_e4m3

B = 128          # batch
IN_F = 512       # in_features
OUT_F = 128      # out_features
KD = 32          # kernel dim
N_CORES = 8
O_PER_CORE = OUT_F // N_CORES        # 16
N_GRP = O_PER_CORE * KD // 128       # 4 o-groups of (4 o x 32 k) partitions
O_PER_GRP = 128 // KD                # 4
JQ = 4                               # j's per PSUM tile / exp instruction
N_QUAD = B // JQ                     # 32
MW = 32                              # matmul M width per j (16 real + 16 zero)
LAG = 2                              # quads between exp and rowsum/colsum


def _plan():
    """Static engine plan for the 512 (q, jj, g) |d| tiles.

    Greedy makespan balance using the TimelineSim engine-busy costs
    (f = 128-4q): DVE 60.4+0.260f, ScalarE 185+0.833f, GpSimd
    95+1.389f.  Fixed loads: DVE carries evictions/upcasts/rowsums,
    ScalarE the exps.  Tiles within a quad are interchangeable, so the
    per-quad engine multiset is then packed into regions with the
    non-DVE tiles concentrated at region 0 (the only DoubleRow-legal
    dst), paired for fp8.

    Returns regions[q][jj] = list of 4 labels in {'D','A','P'} and
    pair8[q] = (pair0_is_fp8, pair1_is_fp8) for region 0.
    """
    load = {
        "D": 4 * 258 + 4 * 127 + 254 + sum(60.4 + 0.260 * (128 - 4 * q)
                                           for q in range(N_QUAD)),
        "A": sum(185 + 0.833 * (128 - 4 * q) for q in range(N_QUAD)),
        "P": 0.0,
    }
    cost = {
        "D": lambda f: 60.4 + 0.260 * f,
        "A": lambda f: 185 + 0.833 * f,
        "P": lambda f: 95 + 1.389 * f,
    }
    counts = [{"D": 0, "A": 0, "P": 0} for _ in range(N_QUAD)]
    tiles = [(128 - 4 * q, q) for q in range(N_QUAD)
             for _ in range(JQ * N_GRP)]
    tiles.sort(key=lambda t: -t[0])
    for f, q in tiles:
        pick = min(cost, key=lambda e: load[e] + cost[e](f))
        load[pick] += cost[pick](f)
        counts[q][pick] += 1
    regions = []
    pair8 = []
    for q in range(N_QUAD):
        c = dict(counts[q])
        nond = ["P"] * c["P"] + ["A"] * c["A"]
        dd = ["D"] * c["D"]
        r0 = [(nond or dd).pop(0) for _ in range(N_GRP)]
        rest = nond + dd
        regs = [r0] + [[rest.pop(0) for _ in range(N_GRP)]
                       for _ in range(1, JQ)]
        pair8.append((r0[0] != "D" and r0[1] != "D",
                      r0[2] != "D" and r0[3] != "D"))
        regions.append(regs)
    return regions, pair8, load


_REG, _PAIR8, _LOAD = _plan()


def _build():
    f32, bf16 = mybir.dt.float32, mybir.dt.bfloat16
    fp8 = mybir.dt.float8e4
    A = mybir.AluOpType
    AF = mybir.ActivationFunctionType
    nc = bacc.Bacc("TRN2", target_bir_lowering=False, debug=False)

    tt_d = nc.dram_tensor("tt", [IN_F, O_PER_CORE * KD], bf16, kind="ExternalInput")
    xt_d = nc.dram_tensor("xt", [IN_F, B], bf16, kind="ExternalInput")
    s2b_d = nc.dram_tensor("s2b", [128, 2, N_GRP, MW], bf16, kind="ExternalInput")
    s8_d = nc.dram_tensor("s8", [128, 2, 4, 2, MW], fp8, kind="ExternalInput")
    sel4_d = nc.dram_tensor("sel4", [128, O_PER_CORE], bf16, kind="ExternalInput")
    id_d = nc.dram_tensor("idm", [128, 128], bf16, kind="ExternalInput")
    c1_d = nc.dram_tensor("c1", [B, N_QUAD, 128], bf16, kind="ExternalInput")
    bq_d = nc.dram_tensor("bq", [128, N_QUAD], f32, kind="ExternalInput")
    rs_d = nc.dram_tensor("rs", [128, N_QUAD], f32, kind="ExternalOutput")
    acc_d = nc.dram_tensor("accs", [O_PER_CORE, B - JQ], f32, kind="ExternalOutput")

    n_chunk = IN_F // 128  # 4 contraction chunks

    with tile.TileContext(nc) as tc:
        with (
            tc.tile_pool(name="singles", bufs=1) as singles,
            tc.tile_pool(name="adpool", bufs=12) as adpool,
            tc.tile_pool(name="a8pool", bufs=8) as a8pool,
            tc.tile_pool(name="epool", bufs=LAG + 3) as epool,
            tc.tile_pool(name="psn", bufs=4, space="PSUM") as psn,
            tc.tile_pool(name="psg", bufs=2, space="PSUM") as psg,
            tc.tile_pool(name="psa", bufs=1, space="PSUM") as psa,
        ):
            # --- warm the ACT exp/abs tables while DMAs run ---
            warm = singles.tile([1, 4], mybir.dt.float32, tag="warm")
            nc.vector.memset(warm[:], 0.0)
            nc.scalar.activation(
                out=warm[0:1, 0:1], in_=warm[0:1, 1:2],
                func=AF.Exp, bias=0.0, scale=-1.0,
            )
            nc.scalar.activation(
                out=warm[0:1, 2:3], in_=warm[0:1, 3:4],
                func=AF.Abs, bias=0.0, scale=-1.0,
            )

            # --- batched input DMAs, all on the SP queue ---
            # tt: dram [512, 512] -> sbuf [128, 4c, 512]
            t_sb = singles.tile([128, n_chunk, O_PER_CORE * KD], bf16, tag="t")
            x_sb = singles.tile([128, n_chunk, B], bf16, tag="x")
            c1_sb = singles.tile([B, N_QUAD, 128], bf16, tag="c1")
            nc.sync.dma_start(
                t_sb[:],
                tt_d.reshape([n_chunk, 128, O_PER_CORE * KD]).transpose([1, 0, 2]),
            )
            nc.sync.dma_start(
                x_sb[:], xt_d.reshape([n_chunk, 128, B]).transpose([1, 0, 2]),
            )
            nc.sync.dma_start(c1_sb[:, 0:8, :], c1_d[:, 0:8, :])
            s2b_sb = singles.tile([128, 2, N_GRP, MW], bf16, tag="s2b")
            nc.sync.dma_start(s2b_sb[:], s2b_d[:])
            s8_sb = singles.tile([128, 2, 4, 2, MW], fp8, tag="s8")
            nc.sync.dma_start(s8_sb[:], s8_d[:])
            sel4_sb = singles.tile([128, O_PER_CORE], bf16, tag="sel4")
            nc.sync.dma_start(sel4_sb[:], sel4_d[:])
            id_sb = singles.tile([128, 128], bf16, tag="idm")
            nc.sync.dma_start(id_sb[:], id_d[:])
            bq_sb = singles.tile([128, N_QUAD], f32, tag="bq")
            nc.sync.dma_start(bq_sb[:], bq_d[:])
            nc.sync.dma_start(c1_sb[:, 8:20, :], c1_d[:, 8:20, :])
            nc.sync.dma_start(c1_sb[:, 20:32, :], c1_d[:, 20:32, :])

            # --- GEMM: M[g] = (T_g)^T x^T : [(4o,32k)=128, i=128] ---
            m_bf = []
            m32 = []
            for g in range(N_GRP):
                pg = psg.tile([128, B], f32, tag="gemm")
                for c in range(n_chunk):
                    nc.tensor.matmul(
                        pg[:],
                        t_sb[:, c, g * 128:(g + 1) * 128],
                        x_sb[:, c, :],
                        start=(c == 0),
                        stop=(c == n_chunk - 1),
                    )
                mb = singles.tile([128, B], bf16, tag=f"mb{g}")
                nc.vector.tensor_copy(mb[:], pg[:])   # PSUM -> SBUF, bf16
                m_bf.append(mb)
                mu = singles.tile([128, B], f32, tag=f"mu{g}")
                nc.vector.tensor_copy(mu[:], mb[:])   # exact f32 upcast
                m32.append(mu)

            # --- pairwise: per j-quad, |d| tiles -> k-reduce -> exp ---
            rs_sb = singles.tile([128, N_QUAD], f32, tag="rs")
            acc_ps = psa.tile([O_PER_CORE, B], f32, tag="accp")
            pending = []

            def emit_tile(eng, dst, g, j, i0):
                if eng == "D":
                    nc.vector.tensor_scalar(
                        out=dst, in0=m_bf[g][:, i0:B],
                        scalar1=m32[g][:, j:j + 1], scalar2=0.0,
                        op0=A.subtract, op1=A.max,
                    )
                elif eng == "A":
                    nc.scalar.activation(
                        out=dst, in_=m_bf[g][:, i0:B],
                        func=AF.Abs,
                        bias=m32[g][:, j:j + 1], scale=-1.0,
                    )
                else:
                    nc.gpsimd.tensor_scalar(
                        out=dst, in0=m_bf[g][:, i0:B],
                        scalar1=m32[g][:, j:j + 1], scalar2=0.0,
                        op0=A.subtract, op1=A.max,
                    )

            def finish(q, e_tile, f):
                nc.vector.tensor_reduce(
                    out=rs_sb[:, q:q + 1], in_=e_tile[:, 0:f],
                    axis=mybir.AxisListType.X, op=A.add,
                )
                if q < N_QUAD - 1:
                    nc.tensor.matmul(
                        acc_ps[:, 4 * q + JQ:B], sel4_sb[:],
                        e_tile[:, JQ:f],
                        start=(q == 0), stop=(q == N_QUAD - 2),
                        skip_group_check=True,
                    )

            for q in range(N_QUAD):
                i0 = 4 * q
                f = 128 - i0
                pn = psn.tile([128, B], f32, tag="norm")
                # seed: pn[row, i] = -P_S(row)[i, o(row)] for all 4 regions
                nc.tensor.matmul(
                    pn[:, 0:f], c1_sb[:, q, :], id_sb[:, i0:B],
                    start=True, stop=False, skip_group_check=True,
                )
                for jj in range(JQ):
                    j = JQ * q + jj
                    labels = _REG[q][jj]
                    reg = pn[MW * jj:MW * (jj + 1), 0:f]
                    mms = []
                    if jj == 0:
                        for pr in range(2):
                            gs = (2 * pr, 2 * pr + 1)
                            if _PAIR8[q][pr]:
                                a8 = a8pool.tile([128, 2, B], fp8, tag="a8")
                                for t, g in enumerate(gs):
                                    emit_tile(labels[g], a8[:, t, 0:f], g, j, i0)
                                # selector variant by (slot0, slot1) weights
                                v = ((labels[gs[0]] == "P") * 2
                                     + (labels[gs[1]] == "P") * 1)
                                mms.append(("dr", pr, v, a8))
                            else:
                                for g in gs:
                                    ad = adpool.tile([128, B], bf16, tag="ad")
                                    emit_tile(labels[g], ad[:, 0:f], g, j, i0)
                                    mms.append(("b", g, labels[g], ad))
                    else:
                        for g in range(N_GRP):
                            ad = adpool.tile([128, B], bf16, tag="ad")
                            emit_tile(labels[g], ad[:, 0:f], g, j, i0)
                            mms.append(("b", g, labels[g], ad))
                    for idx, mm in enumerate(mms):
                        stop = idx == len(mms) - 1
                        if mm[0] == "dr":
                            _, pr, v, a8 = mm
                            nc.tensor.matmul(
                                reg, s8_sb[:, pr, v, :, :], a8[:, :, 0:f],
                                start=False, stop=stop,
                                perf_mode=mybir.MatmulPerfMode.DoubleRow,
                                tile_position=(0, MW * jj),
                                skip_group_check=True,
                            )
                        else:
                            _, g, lab, ad = mm
                            w = 0 if lab != "A" else 1   # 0: weight 2, 1: weight 1
                            nc.tensor.matmul(
                                reg, s2b_sb[:, w, g, :], ad[:, 0:f],
                                start=False, stop=stop,
                                tile_position=(0, MW * jj),
                                skip_group_check=True,
                            )

                e_tile = epool.tile([128, B], bf16, tag="e")
                nc.scalar.activation(
                    out=e_tile[:, 0:f], in_=pn[:, 0:f],
                    func=AF.Exp, bias=bq_sb[:, q:q + 1], scale=-1.0,
                )
                pending.append((q, e_tile, f))
                if len(pending) > LAG:
                    finish(*pending.pop(0))

            while pending:
                finish(*pending.pop(0))

            # --- ship results ---
            acc_sb = singles.tile([O_PER_CORE, B - JQ], f32, tag="acc_sb")
            nc.vector.tensor_copy(acc_sb[:], acc_ps[:, JQ:B])
            nc.sync.dma_start(rs_d[:], rs_sb[:])
            nc.sync.dma_start(acc_d[:], acc_sb[:])

    nc.compile()
    return nc


_NC = None


def kernel(x: np.ndarray, T: np.ndarray) -> np.ndarray:
    global _NC
    if _NC is None:
        _NC = _build()
    nc = _NC

    x = np.ascontiguousarray(x, dtype=np.float32)
    T = np.ascontiguousarray(T, dtype=np.float32)

    xt = np.ascontiguousarray(x.T).astype(BF16)                 # [512, 128]

    # selectors: col g*4 + o_loc, weight 2 (relu tiles) or 1 (abs tiles)
    s2b = np.zeros((128, 2, N_GRP, MW), dtype=BF16)
    for p in range(128):
        o_loc = p // KD
        for g in range(N_GRP):
            s2b[p, 0, g, g * O_PER_GRP + o_loc] = 2
            s2b[p, 1, g, g * O_PER_GRP + o_loc] = 1
    # fp8 DoubleRow selector: pair pr covers groups (2pr, 2pr+1); variant
    # v encodes (slot0_weight==2)*2 + (slot1_weight==2)*1
    s8 = np.zeros((128, 2, 4, 2, MW), dtype=FP8)
    for p in range(128):
        o_loc = p // KD
        for pr in range(2):
            for v in range(4):
                w0 = 2 if v & 2 else 1
                w1 = 2 if v & 1 else 1
                g0, g1 = 2 * pr, 2 * pr + 1
                s8[p, pr, v, 0, g0 * O_PER_GRP + o_loc] = w0
                s8[p, pr, v, 1, g1 * O_PER_GRP + o_loc] = w1
    # colsum selector: partition (jj, c) -> column c (c < 16)
    sel4 = np.zeros((128, O_PER_CORE), dtype=BF16)
    for jj in range(JQ):
        for c in range(O_PER_CORE):
            sel4[MW * jj + c, c] = 1
    ident = np.eye(128, dtype=BF16)

    # host-side P[i, o] = sum_k m[i, o, k]; each o belongs to exactly one
    # group g = (o mod 16) // 4, so the relu correction for row (jj, o)
    # is P[i, o] masked by whether that region's group-g tile is
    # relu-produced (label != 'A').  Only consistency with the device's
    # bf16 m matters (the +P/-P copies cancel exactly on the diagonal).
    m_host = (x @ T.reshape(IN_F, OUT_F * KD)).reshape(B, OUT_F, KD)
    P_all = m_host.sum(axis=-1)                                 # [B, 128]

    in_maps = []
    for core in range(N_CORES):
        t_slice = T[:, core * O_PER_CORE:(core + 1) * O_PER_CORE, :]
        tt = np.ascontiguousarray(
            t_slice.reshape(IN_F, O_PER_CORE * KD)).astype(BF16)
        P = P_all[:, core * O_PER_CORE:(core + 1) * O_PER_CORE]  # [B, 16]
        Pb = P.astype(BF16)                   # bf16-quantized, used as-is
        Pf = Pb.astype(np.float32)
        # c1[i, q, row(jj,c)] = -bf16(P_S); bq[row, q] = -f32(bf16(P_S))[j]
        c1 = np.zeros((B, N_QUAD, 128), dtype=BF16)
        bq = np.zeros((128, N_QUAD), dtype=np.float32)
        for q in range(N_QUAD):
            for jj in range(JQ):
                labels = _REG[q][jj]
                mask = np.array([labels[c // O_PER_GRP] != "A"
                                 for c in range(O_PER_CORE)])
                c1[:, q, MW * jj:MW * jj + O_PER_CORE] = \
                    np.where(mask[None, :], -Pf, 0.0).astype(BF16)
                bq[MW * jj:MW * jj + O_PER_CORE, q] = \
                    np.where(mask, -Pf[JQ * q + jj, :], 0.0)
        in_maps.append({"tt": tt, "xt": xt, "s2b": s2b, "s8": s8,
                        "sel4": sel4, "idm": ident, "c1": c1, "bq": bq})

    res = run_bass_kernel_spmd(nc, in_maps, core_ids=list(range(N_CORES)))

    ob_full = np.empty((B, OUT_F), dtype=np.float32)
    for c, r in enumerate(res.results):
        rs = r["rs"]                                            # [128, 32]
        ac = r["accs"]                                          # [16, 124]
        row = rs.reshape(JQ, MW, N_QUAD)[:, :O_PER_CORE, :]     # [jj, r, q]
        ob = row.transpose(2, 0, 1).reshape(B, O_PER_CORE)      # [j, r]
        ob[JQ:, :] += ac.T                                      # j >= 4
        ob_full[:, c * O_PER_CORE:(c + 1) * O_PER_CORE] = ob
    out = np.concatenate([x, ob_full - 1.0], axis=1).astype(np.float32)
    return out


if __name__ == "__main__":
    print("plan loads (ns):", {k: round(v) for k, v in _LOAD.items()})
    n8 = sum(p[0] + p[1] for p in _PAIR8)
    print(f"fp8 DR pairs: {n8}/64")


# revision 7
# speedup vs baseline: 1.0054x; 1.0054x over previous
"""Minibatch discrimination kernel for 8 Trainium2 NeuronCores.

Reference computation:
    m = (x @ T.reshape(512, 128*32)).reshape(B=128, O=128, K=32)
    norm[i,j,o] = sum_k |m[i,o,k] - m[j,o,k]|
    o_b[j,o]    = sum_i exp(-norm[i,j,o]) - 1
    out         = concat([x, o_b], axis=1)            # [128, 640]

Distribution: shard the output-feature dim O=128 across the 8 cores (16
o's per core); no collectives.  Each core runs the GEMM for its T-slice
and the BxB pairwise exp-sum for its o-slice.

Per-core dataflow (tiles are [partition, free]):
  - GEMM -> M per o-group g as [(4o x 32k)=128 partitions, i=128] in
    PSUM; evicted to bf16 m_bf plus an exact f32 upcast m32 (the
    per-partition scalar / activation bias source).
  - norm is symmetric, so quad q (4 j's) only computes columns
    i >= 4q: free dim shrinks 128 -> 4 across quads, halving the
    elementwise volume.  The missing i < 4q part of o_b comes back via
    per-quad column sums (see below).
  - |d| tiles, one fused op per (j-region, o-group):
      DVE / GpSimd: tensor_scalar(subtract, max, 0) = relu(m_i - m_j)
        (weight-2 selector + P-correction; abs is not in the DVE/Pool
        hw ISA),
      ScalarE: activation Abs(-m + bias m32[:,j]) = |d| directly
        (weight-1 selector, no correction).
    A static plan balances the three engines' busy time.
  - k-reduction on the TensorEngine: per quad one seed matmul deposits
    -P_S[i,o] (P_S = sum over the RELU-produced groups of that row's
    region, host-precomputed per quad in c1) and per tile a selector
    matmul accumulates into the [(4j x 32(16o+16pad)), i] PSUM tile.
    Region jj=0 packs its tiles as fp8 pairs consumed by DoubleRow
    matmuls (0.5 cyc/row; hw requires dst partition base 0, so only
    this region qualifies).
  - One ScalarE Exp per quad with bias +P_S[j,o] (rides the bq table)
    -> E_q in SBUF bf16.  Row sums via DVE free-axis reduce into
    rs[:, q]; column sums via one PE matmul per quad accumulating
    sel4^T @ E_q[:, 4:] into ACC[16 o, j] over columns j >= 4q+4 only,
    so ACC[o,j] ends as sum_{i<4q_j} E[i,j] (prefix by construction).
  - Host combines o_b[j,o] = rs + ACC - 1 and concats with x.
"""

import numpy as np
import ml_dtypes

import concourse.bacc as bacc
import concourse.tile as tile
import concourse.mybir as mybir
from concourse.bass_utils import run_bass_kernel_spmd

BF16 = ml_dtypes.bfloat16
FP8 = ml_dtypes.float8_e4m3

B = 128          # batch
IN_F = 512       # in_features
OUT_F = 128      # out_features
KD = 32          # kernel dim
N_CORES = 8
O_PER_CORE = OUT_F // N_CORES        # 16
N_GRP = O_PER_CORE * KD // 128       # 4 o-groups of (4 o x 32 k) partitions
O_PER_GRP = 128 // KD                # 4
JQ = 4                               # j's per PSUM tile / exp instruction
N_QUAD = B // JQ                     # 32
MW = 32                              # matmul M width per j (16 real + 16 zero)
LAG = 3                              # quads between exp and rowsum/colsum
ELAG = 2                             # quads between norm-psum and exp


def _plan():
    """Static engine plan for the 512 (q, jj, g) |d| tiles.

    Greedy makespan balance using the TimelineSim engine-busy costs
    (f = 128-4q): DVE 60.4+0.260f, ScalarE 185+0.833f, GpSimd
    95+1.389f.  Fixed loads: DVE carries evictions/upcasts/rowsums,
    ScalarE the exps.  Tiles within a quad are interchangeable, so the
    per-quad engine multiset is then packed into regions with the
    non-DVE tiles concentrated at region 0 (the only DoubleRow-legal
    dst), paired for fp8.

    Returns regions[q][jj] = list of 4 labels in {'D','A','P'} and
    pair8[q] = (pair0_is_fp8, pair1_is_fp8) for region 0.
    """
    load = {
        "D": 4 * 258 + 4 * 127 + 254 + sum(60.4 + 0.260 * (128 - 4 * q)
                                           for q in range(N_QUAD)),
        "A": sum(185 + 0.833 * (128 - 4 * q) for q in range(N_QUAD)),
        "P": 0.0,
    }
    cost = {
        "D": lambda f: 60.4 + 0.260 * f,
        "A": lambda f: 185 + 0.833 * f,
        "P": lambda f: 95 + 1.389 * f,
    }
    counts = [{"D": 0, "A": 0, "P": 0} for _ in range(N_QUAD)]
    tiles = [(128 - 4 * q, q) for q in range(N_QUAD)
             for _ in range(JQ * N_GRP)]
    tiles.sort(key=lambda t: -t[0])
    for f, q in tiles:
        pick = min(cost, key=lambda e: load[e] + cost[e](f))
        load[pick] += cost[pick](f)
        counts[q][pick] += 1
    regions = []
    pair8 = []
    for q in range(N_QUAD):
        c = dict(counts[q])
        nond = ["P"] * c["P"] + ["A"] * c["A"]
        dd = ["D"] * c["D"]
        r0 = [(nond or dd).pop(0) for _ in range(N_GRP)]
        rest = nond + dd
        regs = [r0] + [[rest.pop(0) for _ in range(N_GRP)]
                       for _ in range(1, JQ)]
        pair8.append((r0[0] != "D" and r0[1] != "D",
                      r0[2] != "D" and r0[3] != "D"))
        regions.append(regs)
    return regions, pair8, load


_REG, _PAIR8, _LOAD = _plan()


def _build():
    f32, bf16 = mybir.dt.float32, mybir.dt.bfloat16
    fp8 = mybir.dt.float8e4
    A = mybir.AluOpType
    AF = mybir.ActivationFunctionType
    nc = bacc.Bacc("TRN2", target_bir_lowering=False, debug=False)

    tt_d = nc.dram_tensor("tt", [IN_F, O_PER_CORE * KD], bf16, kind="ExternalInput")
    xt_d = nc.dram_tensor("xt", [IN_F, B], bf16, kind="ExternalInput")
    s2b_d = nc.dram_tensor("s2b", [128, 2, N_GRP, MW], bf16, kind="ExternalInput")
    s8_d = nc.dram_tensor("s8", [128, 2, 4, 2, MW], fp8, kind="ExternalInput")
    sel4_d = nc.dram_tensor("sel4", [128, O_PER_CORE], bf16, kind="ExternalInput")
    id_d = nc.dram_tensor("idm", [128, 128], bf16, kind="ExternalInput")
    c1_d = nc.dram_tensor("c1", [B, N_QUAD, 128], bf16, kind="ExternalInput")
    bq_d = nc.dram_tensor("bq", [128, N_QUAD], f32, kind="ExternalInput")
    rs_d = nc.dram_tensor("rs", [128, N_QUAD], f32, kind="ExternalOutput")
    acc_d = nc.dram_tensor("accs", [O_PER_CORE, B - JQ], f32, kind="ExternalOutput")

    n_chunk = IN_F // 128  # 4 contraction chunks

    with tile.TileContext(nc) as tc:
        with (
            tc.tile_pool(name="singles", bufs=1) as singles,
            tc.tile_pool(name="adpool", bufs=48) as adpool,
            tc.tile_pool(name="a8pool", bufs=16) as a8pool,
            tc.tile_pool(name="epool", bufs=LAG + 3) as epool,
            tc.tile_pool(name="psn", bufs=5, space="PSUM") as psn,
            tc.tile_pool(name="psa", bufs=1, space="PSUM") as psa,
        ):
            # --- warm the ACT exp/abs tables while DMAs run ---
            warm = singles.tile([1, 4], mybir.dt.float32, tag="warm")
            nc.vector.memset(warm[:], 0.0)
            nc.scalar.activation(
                out=warm[0:1, 0:1], in_=warm[0:1, 1:2],
                func=AF.Exp, bias=0.0, scale=-1.0,
            )
            nc.scalar.activation(
                out=warm[0:1, 2:3], in_=warm[0:1, 3:4],
                func=AF.Abs, bias=0.0, scale=-1.0,
            )

            # --- batched input DMAs, all on the SP queue ---
            # tt: dram [512, 512] -> sbuf [128, 4c, 512]
            t_sb = singles.tile([128, n_chunk, O_PER_CORE * KD], bf16, tag="t")
            x_sb = singles.tile([128, n_chunk, B], bf16, tag="x")
            c1_sb = singles.tile([B, N_QUAD, 128], bf16, tag="c1")
            nc.sync.dma_start(
                t_sb[:],
                tt_d.reshape([n_chunk, 128, O_PER_CORE * KD]).transpose([1, 0, 2]),
            )
            nc.sync.dma_start(
                x_sb[:], xt_d.reshape([n_chunk, 128, B]).transpose([1, 0, 2]),
            )
            nc.gpsimd.dma_start(c1_sb[:, 0:8, :], c1_d[:, 0:8, :])
            s2b_sb = singles.tile([128, 2, N_GRP, MW], bf16, tag="s2b")
            nc.sync.dma_start(s2b_sb[:], s2b_d[:])
            s8_sb = singles.tile([128, 2, 4, 2, MW], fp8, tag="s8")
            nc.gpsimd.dma_start(s8_sb[:], s8_d[:])
            sel4_sb = singles.tile([128, O_PER_CORE], bf16, tag="sel4")
            nc.sync.dma_start(sel4_sb[:], sel4_d[:])
            id_sb = singles.tile([128, 128], bf16, tag="idm")
            nc.sync.dma_start(id_sb[:], id_d[:])
            bq_sb = singles.tile([128, N_QUAD], f32, tag="bq")
            nc.gpsimd.dma_start(bq_sb[:], bq_d[:])
            nc.gpsimd.dma_start(c1_sb[:, 8:20, :], c1_d[:, 8:20, :])
            nc.sync.dma_start(c1_sb[:, 20:32, :], c1_d[:, 20:32, :])

            # --- GEMM: M[g] = (T_g)^T x^T : [(4o,32k)=128, i=128] ---
            m_bf = []
            m32 = []
            gemm_pool_cm = tc.tile_pool(name="psg", bufs=2, space="PSUM")
            psg = gemm_pool_cm.__enter__()
            for g in range(N_GRP):
                pg = psg.tile([128, B], f32, tag="gemm")
                for c in range(n_chunk):
                    nc.tensor.matmul(
                        pg[:],
                        t_sb[:, c, g * 128:(g + 1) * 128],
                        x_sb[:, c, :],
                        start=(c == 0),
                        stop=(c == n_chunk - 1),
                    )
                mb = singles.tile([128, B], bf16, tag=f"mb{g}")
                nc.vector.tensor_copy(mb[:], pg[:])   # PSUM -> SBUF, bf16
                m_bf.append(mb)
                mu = singles.tile([128, B], f32, tag=f"mu{g}")
                nc.vector.tensor_copy(mu[:], mb[:])   # exact f32 upcast
                m32.append(mu)
            gemm_pool_cm.__exit__(None, None, None)

            # --- pairwise: per j-quad, |d| tiles -> k-reduce -> exp ---
            rs_sb = singles.tile([128, N_QUAD], f32, tag="rs")
            acc_ps = psa.tile([O_PER_CORE, B], f32, tag="accp")
            pending = []

            def emit_tile(eng, dst, g, j, i0):
                if eng == "D":
                    nc.vector.tensor_scalar(
                        out=dst, in0=m_bf[g][:, i0:B],
                        scalar1=m32[g][:, j:j + 1], scalar2=0.0,
                        op0=A.subtract, op1=A.max,
                    )
                elif eng == "A":
                    nc.scalar.activation(
                        out=dst, in_=m_bf[g][:, i0:B],
                        func=AF.Abs,
                        bias=m32[g][:, j:j + 1], scale=-1.0,
                    )
                else:
                    nc.gpsimd.tensor_scalar(
                        out=dst, in0=m_bf[g][:, i0:B],
                        scalar1=m32[g][:, j:j + 1], scalar2=0.0,
                        op0=A.subtract, op1=A.max,
                    )

            exp_pending = []

            def emit_exp(q, pn_tile, f):
                e_tile = epool.tile([128, B], bf16, tag="e")
                nc.scalar.activation(
                    out=e_tile[:, 0:f], in_=pn_tile[:, 0:f],
                    func=AF.Exp, bias=bq_sb[:, q:q + 1], scale=-1.0,
                )
                pending.append((q, e_tile, f))
                if len(pending) > LAG:
                    finish(*pending.pop(0))

            def finish(q, e_tile, f):
                nc.vector.tensor_reduce(
                    out=rs_sb[:, q:q + 1], in_=e_tile[:, 0:f],
                    axis=mybir.AxisListType.X, op=A.add,
                )
                if q < N_QUAD - 1:
                    nc.tensor.matmul(
                        acc_ps[:, 4 * q + JQ:B], sel4_sb[:],
                        e_tile[:, JQ:f],
                        start=(q == 0), stop=(q == N_QUAD - 2),
                        skip_group_check=True,
                    )

            for q in range(N_QUAD):
                i0 = 4 * q
                f = 128 - i0
                pn = psn.tile([128, B], f32, tag="norm")
                # seed: pn[row, i] = -P_S(row)[i, o(row)] for all 4 regions
                nc.tensor.matmul(
                    pn[:, 0:f], c1_sb[:, q, :], id_sb[:, i0:B],
                    start=True, stop=False, skip_group_check=True,
                )
                for jj in range(JQ):
                    j = JQ * q + jj
                    labels = _REG[q][jj]
                    reg = pn[MW * jj:MW * (jj + 1), 0:f]
                    mms = []
                    if jj == 0:
                        for pr in range(2):
                            gs = (2 * pr, 2 * pr + 1)
                            if _PAIR8[q][pr]:
                                a8 = a8pool.tile([128, 2, B], fp8, tag="a8")
                                for t, g in enumerate(gs):
                                    emit_tile(labels[g], a8[:, t, 0:f], g, j, i0)
                                # selector variant by (slot0, slot1) weights
                                v = ((labels[gs[0]] == "P") * 2
                                     + (labels[gs[1]] == "P") * 1)
                                mms.append(("dr", pr, v, a8))
                            else:
                                for g in gs:
                                    ad = adpool.tile([128, B], bf16, tag="ad")
                                    emit_tile(labels[g], ad[:, 0:f], g, j, i0)
                                    mms.append(("b", g, labels[g], ad))
                    else:
                        for g in range(N_GRP):
                            ad = adpool.tile([128, B], bf16, tag="ad")
                            emit_tile(labels[g], ad[:, 0:f], g, j, i0)
                            mms.append(("b", g, labels[g], ad))
                    for idx, mm in enumerate(mms):
                        stop = idx == len(mms) - 1
                        if mm[0] == "dr":
                            _, pr, v, a8 = mm
                            nc.tensor.matmul(
                                reg, s8_sb[:, pr, v, :, :], a8[:, :, 0:f],
                                start=False, stop=stop,
                                perf_mode=mybir.MatmulPerfMode.DoubleRow,
                                tile_position=(0, MW * jj),
                                skip_group_check=True,
                            )
                        else:
                            _, g, lab, ad = mm
                            w = 0 if lab != "A" else 1   # 0: weight 2, 1: weight 1
                            nc.tensor.matmul(
                                reg, s2b_sb[:, w, g, :], ad[:, 0:f],
                                start=False, stop=stop,
                                tile_position=(0, MW * jj),
                                skip_group_check=True,
                            )

                exp_pending.append((q, pn, f))
                if len(exp_pending) > ELAG:
                    emit_exp(*exp_pending.pop(0))

            while exp_pending:
                emit_exp(*exp_pending.pop(0))
            while pending:
                finish(*pending.pop(0))

            # --- ship results ---
            acc_sb = singles.tile([O_PER_CORE, B - JQ], f32, tag="acc_sb")
            nc.vector.tensor_copy(acc_sb[:], acc_ps[:, JQ:B])
            nc.sync.dma_start(rs_d[:], rs_sb[:])
            nc.sync.dma_start(acc_d[:], acc_sb[:])

    nc.compile()
    return nc


_NC = None


def kernel(x: np.ndarray, T: np.ndarray) -> np.ndarray:
    global _NC
    if _NC is None:
        _NC = _build()
    nc = _NC

    x = np.ascontiguousarray(x, dtype=np.float32)
    T = np.ascontiguousarray(T, dtype=np.float32)

    xt = np.ascontiguousarray(x.T).astype(BF16)                 # [512, 128]

    # selectors: col g*4 + o_loc, weight 2 (relu tiles) or 1 (abs tiles)
    s2b = np.zeros((128, 2, N_GRP, MW), dtype=BF16)
    for p in range(128):
        o_loc = p // KD
        for g in range(N_GRP):
            s2b[p, 0, g, g * O_PER_GRP + o_loc] = 2
            s2b[p, 1, g, g * O_PER_GRP + o_loc] = 1
    # fp8 DoubleRow selector: pair pr covers groups (2pr, 2pr+1); variant
    # v encodes (slot0_weight==2)*2 + (slot1_weight==2)*1
    s8 = np.zeros((128, 2, 4, 2, MW), dtype=FP8)
    for p in range(128):
        o_loc = p // KD
        for pr in range(2):
            for v in range(4):
                w0 = 2 if v & 2 else 1
                w1 = 2 if v & 1 else 1
                g0, g1 = 2 * pr, 2 * pr + 1
                s8[p, pr, v, 0, g0 * O_PER_GRP + o_loc] = w0
                s8[p, pr, v, 1, g1 * O_PER_GRP + o_loc] = w1
    # colsum selector: partition (jj, c) -> column c (c < 16)
    sel4 = np.zeros((128, O_PER_CORE), dtype=BF16)
    for jj in range(JQ):
        for c in range(O_PER_CORE):
            sel4[MW * jj + c, c] = 1
    ident = np.eye(128, dtype=BF16)

    # host-side P[i, o] = sum_k m[i, o, k]; each o belongs to exactly one
    # group g = (o mod 16) // 4, so the relu correction for row (jj, o)
    # is P[i, o] masked by whether that region's group-g tile is
    # relu-produced (label != 'A').  Only consistency with the device's
    # bf16 m matters (the +P/-P copies cancel exactly on the diagonal).
    m_host = (x @ T.reshape(IN_F, OUT_F * KD)).reshape(B, OUT_F, KD)
    P_all = m_host.sum(axis=-1)                                 # [B, 128]

    in_maps = []
    for core in range(N_CORES):
        t_slice = T[:, core * O_PER_CORE:(core + 1) * O_PER_CORE, :]
        tt = np.ascontiguousarray(
            t_slice.reshape(IN_F, O_PER_CORE * KD)).astype(BF16)
        P = P_all[:, core * O_PER_CORE:(core + 1) * O_PER_CORE]  # [B, 16]
        Pb = P.astype(BF16)                   # bf16-quantized, used as-is
        Pf = Pb.astype(np.float32)
        # c1[i, q, row(jj,c)] = -bf16(P_S); bq[row, q] = -f32(bf16(P_S))[j]
        c1 = np.zeros((B, N_QUAD, 128), dtype=BF16)
        bq = np.zeros((128, N_QUAD), dtype=np.float32)
        for q in range(N_QUAD):
            for jj in range(JQ):
                labels = _REG[q][jj]
                mask = np.array([labels[c // O_PER_GRP] != "A"
                                 for c in range(O_PER_CORE)])
                c1[:, q, MW * jj:MW * jj + O_PER_CORE] = \
                    np.where(mask[None, :], -Pf, 0.0).astype(BF16)
                bq[MW * jj:MW * jj + O_PER_CORE, q] = \
                    np.where(mask, -Pf[JQ * q + jj, :], 0.0)
        in_maps.append({"tt": tt, "xt": xt, "s2b": s2b, "s8": s8,
                        "sel4": sel4, "idm": ident, "c1": c1, "bq": bq})

    res = run_bass_kernel_spmd(nc, in_maps, core_ids=list(range(N_CORES)))

    ob_full = np.empty((B, OUT_F), dtype=np.float32)
    for c, r in enumerate(res.results):
        rs = r["rs"]                                            # [128, 32]
        ac = r["accs"]                                          # [16, 124]
        row = rs.reshape(JQ, MW, N_QUAD)[:, :O_PER_CORE, :]     # [jj, r, q]
        ob = row.transpose(2, 0, 1).reshape(B, O_PER_CORE)      # [j, r]
        ob[JQ:, :] += ac.T                                      # j >= 4
        ob_full[:, c * O_PER_CORE:(c + 1) * O_PER_CORE] = ob
    out = np.concatenate([x, ob_full - 1.0], axis=1).astype(np.float32)
    return out


if __name__ == "__main__":
    print("plan loads (ns):", {k: round(v) for k, v in _LOAD.items()})
    n8 = sum(p[0] + p[1] for p in _PAIR8)
    print(f"fp8 DR pairs: {n8}/64")


# revision 9
# speedup vs baseline: 1.0283x; 1.0227x over previous
"""Minibatch discrimination kernel for 8 Trainium2 NeuronCores.

Reference computation:
    m = (x @ T.reshape(512, 128*32)).reshape(B=128, O=128, K=32)
    norm[i,j,o] = sum_k |m[i,o,k] - m[j,o,k]|
    o_b[j,o]    = sum_i exp(-norm[i,j,o]) - 1
    out         = concat([x, o_b], axis=1)            # [128, 640]

Distribution: shard the output-feature dim O=128 across the 8 cores (16
o's per core); no collectives.  Each core runs the GEMM for its T-slice
and the BxB pairwise exp-sum for its o-slice.

Per-core dataflow (tiles are [partition, free]):
  - GEMM -> M per o-group g as [(4o x 32k)=128 partitions, i=128] in
    PSUM; evicted to bf16 m_bf plus an exact f32 upcast m32 (the
    per-partition scalar / activation bias source).
  - norm is symmetric, so quad q (4 j's) only computes columns
    i >= 4q: free dim shrinks 128 -> 4 across quads, halving the
    elementwise volume.  The missing i < 4q part of o_b comes back via
    per-quad column sums (see below).
  - |d| tiles, one fused op per (j-region, o-group):
      DVE / GpSimd: tensor_scalar(subtract, max, 0) = relu(m_i - m_j)
        (weight-2 selector + P-correction; abs is not in the DVE/Pool
        hw ISA),
      ScalarE: activation Abs(-m + bias m32[:,j]) = |d| directly
        (weight-1 selector, no correction).
    A static plan balances the three engines' busy time.
  - k-reduction on the TensorEngine: per quad one seed matmul deposits
    -P_S[i,o] (P_S = sum over the RELU-produced groups of that row's
    region, host-precomputed per quad in c1) and per tile a selector
    matmul accumulates into the [(4j x 32(16o+16pad)), i] PSUM tile.
    Region jj=0 packs its tiles as fp8 pairs consumed by DoubleRow
    matmuls (0.5 cyc/row; hw requires dst partition base 0, so only
    this region qualifies).
  - One ScalarE Exp per quad with bias +P_S[j,o] (rides the bq table)
    -> E_q in SBUF bf16.  Row sums via DVE free-axis reduce into
    rs[:, q]; column sums via one PE matmul per quad accumulating
    sel4^T @ E_q[:, 4:] into ACC[16 o, j] over columns j >= 4q+4 only,
    so ACC[o,j] ends as sum_{i<4q_j} E[i,j] (prefix by construction).
  - Host combines o_b[j,o] = rs + ACC - 1 and concats with x.
"""

import numpy as np
import ml_dtypes

import concourse.bacc as bacc
import concourse.tile as tile
import concourse.mybir as mybir
from concourse.bass_utils import run_bass_kernel_spmd

BF16 = ml_dtypes.bfloat16
FP8 = ml_dtypes.float8_e4m3

B = 128          # batch
IN_F = 512       # in_features
OUT_F = 128      # out_features
KD = 32          # kernel dim
N_CORES = 8
O_PER_CORE = OUT_F // N_CORES        # 16
N_GRP = O_PER_CORE * KD // 128       # 4 o-groups of (4 o x 32 k) partitions
O_PER_GRP = 128 // KD                # 4
JQ = 4                               # j's per PSUM tile / exp instruction
N_QUAD = B // JQ                     # 32
MW = 32                              # matmul M width per j (16 real + 16 zero)
LAG = 3                              # quads between exp and rowsum/colsum
ELAG = 2                             # quads between norm-psum and exp


def _plan():
    """Static engine plan for the 512 (q, jj, g) |d| tiles.

    Greedy makespan balance using the TimelineSim engine-busy costs
    (f = 128-4q): DVE 60.4+0.260f, ScalarE 185+0.833f, GpSimd
    95+1.389f.  Fixed loads: DVE carries evictions/upcasts/rowsums,
    ScalarE the exps.  Tiles within a quad are interchangeable, so the
    per-quad engine multiset is then packed into regions with the
    non-DVE tiles concentrated at region 0 (the only DoubleRow-legal
    dst), paired for fp8.

    Returns regions[q][jj] = list of 4 labels in {'D','A','P'} and
    pair8[q] = (pair0_is_fp8, pair1_is_fp8) for region 0.
    """
    load = {
        "D": 4 * 258 + 4 * 127 + 254 + sum(60.4 + 0.260 * (128 - 4 * q)
                                           for q in range(N_QUAD)),
        "A": sum(185 + 0.833 * (128 - 4 * q) for q in range(N_QUAD)),
        "P": 0.0,
    }
    cost = {
        "D": lambda f: 60.4 + 0.260 * f,
        "A": lambda f: 185 + 0.833 * f,
        "P": lambda f: 95 + 1.389 * f,
    }
    counts = [{"D": 0, "A": 0, "P": 0} for _ in range(N_QUAD)]
    tiles = [(128 - 4 * q, q) for q in range(N_QUAD)
             for _ in range(JQ * N_GRP)]
    tiles.sort(key=lambda t: -t[0])
    for f, q in tiles:
        pick = min(cost, key=lambda e: load[e] + cost[e](f))
        load[pick] += cost[pick](f)
        counts[q][pick] += 1
    regions = []
    pair8 = []
    for q in range(N_QUAD):
        c = dict(counts[q])
        nond = ["P"] * c["P"] + ["A"] * c["A"]
        dd = ["D"] * c["D"]
        r0 = [(nond or dd).pop(0) for _ in range(N_GRP)]
        rest = nond + dd
        regs = [r0] + [[rest.pop(0) for _ in range(N_GRP)]
                       for _ in range(1, JQ)]
        pair8.append((r0[0] != "D" and r0[1] != "D",
                      r0[2] != "D" and r0[3] != "D"))
        regions.append(regs)
    return regions, pair8, load


_REG, _PAIR8, _LOAD = _plan()


def _build():
    f32, bf16 = mybir.dt.float32, mybir.dt.bfloat16
    fp8 = mybir.dt.float8e4
    A = mybir.AluOpType
    AF = mybir.ActivationFunctionType
    nc = bacc.Bacc("TRN2", target_bir_lowering=False, debug=False)

    tt_d = nc.dram_tensor("tt", [N_GRP, 128, IN_F // 128, 128], bf16, kind="ExternalInput")
    xt_d = nc.dram_tensor("xt", [128, IN_F // 128, B], bf16, kind="ExternalInput")
    s2b_d = nc.dram_tensor("s2b", [128, 2, N_GRP, MW], bf16, kind="ExternalInput")
    s8_d = nc.dram_tensor("s8", [128, 2, 4, 2, MW], fp8, kind="ExternalInput")
    sel4_d = nc.dram_tensor("sel4", [128, O_PER_CORE], bf16, kind="ExternalInput")
    id_d = nc.dram_tensor("idm", [128, 128], bf16, kind="ExternalInput")
    c1_d = nc.dram_tensor("c1", [B, N_QUAD, 128], bf16, kind="ExternalInput")
    bq_d = nc.dram_tensor("bq", [128, N_QUAD], f32, kind="ExternalInput")
    rs_d = nc.dram_tensor("rs", [128, N_QUAD], f32, kind="ExternalOutput")
    acc_d = nc.dram_tensor("accs", [O_PER_CORE, B - JQ], f32, kind="ExternalOutput")

    n_chunk = IN_F // 128  # 4 contraction chunks

    with tile.TileContext(nc) as tc:
        with (
            tc.tile_pool(name="singles", bufs=1) as singles,
            tc.tile_pool(name="adpool", bufs=48) as adpool,
            tc.tile_pool(name="a8pool", bufs=16) as a8pool,
            tc.tile_pool(name="epool", bufs=LAG + 3) as epool,
            tc.tile_pool(name="psn", bufs=5, space="PSUM") as psn,
            tc.tile_pool(name="psa", bufs=1, space="PSUM") as psa,
        ):
            # --- warm the ACT exp/abs tables while DMAs run ---
            warm = singles.tile([1, 4], mybir.dt.float32, tag="warm")
            nc.vector.memset(warm[:], 0.0)
            nc.scalar.activation(
                out=warm[0:1, 0:1], in_=warm[0:1, 1:2],
                func=AF.Exp, bias=0.0, scale=-1.0,
            )
            nc.scalar.activation(
                out=warm[0:1, 2:3], in_=warm[0:1, 3:4],
                func=AF.Abs, bias=0.0, scale=-1.0,
            )

            # --- batched input DMAs, all on the SP queue ---
            # tt: dram [512, 512] -> sbuf [128, 4c, 512]
            t_sb = singles.tile([128, N_GRP, n_chunk, 128], bf16, tag="t")
            x_sb = singles.tile([128, n_chunk, B], bf16, tag="x")
            c1_sb = singles.tile([B, N_QUAD, 128], bf16, tag="c1")
            nc.sync.dma_start(t_sb[:, 0, :, :], tt_d[0].transpose([0, 1, 2]))
            nc.sync.dma_start(x_sb[:], xt_d[:])
            nc.sync.dma_start(t_sb[:, 1, :, :], tt_d[1])
            nc.sync.dma_start(t_sb[:, 2, :, :], tt_d[2])
            nc.sync.dma_start(t_sb[:, 3, :, :], tt_d[3])
            nc.gpsimd.dma_start(c1_sb[:, 0:8, :], c1_d[:, 0:8, :])
            s2b_sb = singles.tile([128, 2, N_GRP, MW], bf16, tag="s2b")
            nc.sync.dma_start(s2b_sb[:], s2b_d[:])
            s8_sb = singles.tile([128, 2, 4, 2, MW], fp8, tag="s8")
            nc.gpsimd.dma_start(s8_sb[:], s8_d[:])
            sel4_sb = singles.tile([128, O_PER_CORE], bf16, tag="sel4")
            nc.sync.dma_start(sel4_sb[:], sel4_d[:])
            id_sb = singles.tile([128, 128], bf16, tag="idm")
            nc.sync.dma_start(id_sb[:], id_d[:])
            bq_sb = singles.tile([128, N_QUAD], f32, tag="bq")
            nc.gpsimd.dma_start(bq_sb[:], bq_d[:])
            nc.gpsimd.dma_start(c1_sb[:, 8:20, :], c1_d[:, 8:20, :])
            nc.sync.dma_start(c1_sb[:, 20:32, :], c1_d[:, 20:32, :])

            # --- GEMM: M[g] = (T_g)^T x^T : [(4o,32k)=128, i=128] ---
            m_bf = []
            m32 = []
            gemm_pool_cm = tc.tile_pool(name="psg", bufs=2, space="PSUM")
            psg = gemm_pool_cm.__enter__()
            for g in range(N_GRP):
                pg = psg.tile([128, B], f32, tag="gemm")
                for c in range(n_chunk):
                    nc.tensor.matmul(
                        pg[:],
                        t_sb[:, g, c, :],
                        x_sb[:, c, :],
                        start=(c == 0),
                        stop=(c == n_chunk - 1),
                    )
                mb = singles.tile([128, B], bf16, tag=f"mb{g}")
                nc.vector.tensor_copy(mb[:], pg[:])   # PSUM -> SBUF, bf16
                m_bf.append(mb)
                mu = singles.tile([128, B], f32, tag=f"mu{g}")
                nc.vector.tensor_copy(mu[:], mb[:])   # exact f32 upcast
                m32.append(mu)
            gemm_pool_cm.__exit__(None, None, None)

            # --- pairwise: per j-quad, |d| tiles -> k-reduce -> exp ---
            rs_sb = singles.tile([128, N_QUAD], f32, tag="rs")
            acc_ps = psa.tile([O_PER_CORE, B], f32, tag="accp")
            pending = []

            def emit_tile(eng, dst, g, j, i0):
                if eng == "D":
                    nc.vector.tensor_scalar(
                        out=dst, in0=m_bf[g][:, i0:B],
                        scalar1=m32[g][:, j:j + 1], scalar2=0.0,
                        op0=A.subtract, op1=A.max,
                    )
                elif eng == "A":
                    nc.scalar.activation(
                        out=dst, in_=m_bf[g][:, i0:B],
                        func=AF.Abs,
                        bias=m32[g][:, j:j + 1], scale=-1.0,
                    )
                else:
                    nc.gpsimd.tensor_scalar(
                        out=dst, in0=m_bf[g][:, i0:B],
                        scalar1=m32[g][:, j:j + 1], scalar2=0.0,
                        op0=A.subtract, op1=A.max,
                    )

            exp_pending = []

            def emit_exp(q, pn_tile, f):
                e_tile = epool.tile([128, B], bf16, tag="e")
                nc.scalar.activation(
                    out=e_tile[:, 0:f], in_=pn_tile[:, 0:f],
                    func=AF.Exp, bias=bq_sb[:, q:q + 1], scale=-1.0,
                )
                pending.append((q, e_tile, f))
                if len(pending) > LAG:
                    finish(*pending.pop(0))

            def finish(q, e_tile, f):
                nc.vector.tensor_reduce(
                    out=rs_sb[:, q:q + 1], in_=e_tile[:, 0:f],
                    axis=mybir.AxisListType.X, op=A.add,
                )
                if q == 15:
                    nc.sync.dma_start(rs_d[:, 0:16], rs_sb[:, 0:16])
                elif q == 27:
                    nc.sync.dma_start(rs_d[:, 16:28], rs_sb[:, 16:28])
                if q < N_QUAD - 1:
                    nc.tensor.matmul(
                        acc_ps[:, 4 * q + JQ:B], sel4_sb[:],
                        e_tile[:, JQ:f],
                        start=(q == 0), stop=(q == N_QUAD - 2),
                        skip_group_check=True,
                    )

            for q in range(N_QUAD):
                i0 = 4 * q
                f = 128 - i0
                pn = psn.tile([128, B], f32, tag="norm")
                # seed: pn[row, i] = -P_S(row)[i, o(row)] for all 4 regions
                nc.tensor.matmul(
                    pn[:, 0:f], c1_sb[:, q, :], id_sb[:, i0:B],
                    start=True, stop=False, skip_group_check=True,
                )
                for jj in range(JQ):
                    j = JQ * q + jj
                    labels = _REG[q][jj]
                    reg = pn[MW * jj:MW * (jj + 1), 0:f]
                    mms = []
                    if jj == 0:
                        for pr in range(2):
                            gs = (2 * pr, 2 * pr + 1)
                            if _PAIR8[q][pr]:
                                a8 = a8pool.tile([128, 2, B], fp8, tag="a8")
                                for t, g in enumerate(gs):
                                    emit_tile(labels[g], a8[:, t, 0:f], g, j, i0)
                                # selector variant by (slot0, slot1) weights
                                v = ((labels[gs[0]] == "P") * 2
                                     + (labels[gs[1]] == "P") * 1)
                                mms.append(("dr", pr, v, a8))
                            else:
                                for g in gs:
                                    ad = adpool.tile([128, B], bf16, tag="ad")
                                    emit_tile(labels[g], ad[:, 0:f], g, j, i0)
                                    mms.append(("b", g, labels[g], ad))
                    else:
                        for g in range(N_GRP):
                            ad = adpool.tile([128, B], bf16, tag="ad")
                            emit_tile(labels[g], ad[:, 0:f], g, j, i0)
                            mms.append(("b", g, labels[g], ad))
                    for idx, mm in enumerate(mms):
                        stop = idx == len(mms) - 1
                        if mm[0] == "dr":
                            _, pr, v, a8 = mm
                            nc.tensor.matmul(
                                reg, s8_sb[:, pr, v, :, :], a8[:, :, 0:f],
                                start=False, stop=stop,
                                perf_mode=mybir.MatmulPerfMode.DoubleRow,
                                tile_position=(0, MW * jj),
                                skip_group_check=True,
                            )
                        else:
                            _, g, lab, ad = mm
                            w = 0 if lab != "A" else 1   # 0: weight 2, 1: weight 1
                            nc.tensor.matmul(
                                reg, s2b_sb[:, w, g, :], ad[:, 0:f],
                                start=False, stop=stop,
                                tile_position=(0, MW * jj),
                                skip_group_check=True,
                            )

                exp_pending.append((q, pn, f))
                if len(exp_pending) > ELAG:
                    emit_exp(*exp_pending.pop(0))

            while exp_pending:
                emit_exp(*exp_pending.pop(0))
            while pending:
                finish(*pending.pop(0))

            # --- ship results ---
            acc_sb = singles.tile([O_PER_CORE, B - JQ], f32, tag="acc_sb")
            nc.vector.tensor_copy(acc_sb[:], acc_ps[:, JQ:B])
            nc.sync.dma_start(rs_d[:, 28:N_QUAD], rs_sb[:, 28:N_QUAD])
            nc.sync.dma_start(acc_d[:], acc_sb[:])

    nc.compile()
    return nc


_NC = None


def kernel(x: np.ndarray, T: np.ndarray) -> np.ndarray:
    global _NC
    if _NC is None:
        _NC = _build()
    nc = _NC

    x = np.ascontiguousarray(x, dtype=np.float32)
    T = np.ascontiguousarray(T, dtype=np.float32)

    xt = np.ascontiguousarray(
        x.T.reshape(IN_F // 128, 128, B).transpose(1, 0, 2)).astype(BF16)

    # selectors: col g*4 + o_loc, weight 2 (relu tiles) or 1 (abs tiles)
    s2b = np.zeros((128, 2, N_GRP, MW), dtype=BF16)
    for p in range(128):
        o_loc = p // KD
        for g in range(N_GRP):
            s2b[p, 0, g, g * O_PER_GRP + o_loc] = 2
            s2b[p, 1, g, g * O_PER_GRP + o_loc] = 1
    # fp8 DoubleRow selector: pair pr covers groups (2pr, 2pr+1); variant
    # v encodes (slot0_weight==2)*2 + (slot1_weight==2)*1
    s8 = np.zeros((128, 2, 4, 2, MW), dtype=FP8)
    for p in range(128):
        o_loc = p // KD
        for pr in range(2):
            for v in range(4):
                w0 = 2 if v & 2 else 1
                w1 = 2 if v & 1 else 1
                g0, g1 = 2 * pr, 2 * pr + 1
                s8[p, pr, v, 0, g0 * O_PER_GRP + o_loc] = w0
                s8[p, pr, v, 1, g1 * O_PER_GRP + o_loc] = w1
    # colsum selector: partition (jj, c) -> column c (c < 16)
    sel4 = np.zeros((128, O_PER_CORE), dtype=BF16)
    for jj in range(JQ):
        for c in range(O_PER_CORE):
            sel4[MW * jj + c, c] = 1
    ident = np.eye(128, dtype=BF16)

    # host-side P[i, o] = sum_k m[i, o, k]; each o belongs to exactly one
    # group g = (o mod 16) // 4, so the relu correction for row (jj, o)
    # is P[i, o] masked by whether that region's group-g tile is
    # relu-produced (label != 'A').  Only consistency with the device's
    # bf16 m matters (the +P/-P copies cancel exactly on the diagonal).
    m_host = (x @ T.reshape(IN_F, OUT_F * KD)).reshape(B, OUT_F, KD)
    P_all = m_host.sum(axis=-1)                                 # [B, 128]

    in_maps = []
    for core in range(N_CORES):
        t_slice = T[:, core * O_PER_CORE:(core + 1) * O_PER_CORE, :]
        # [g][p=(4o,32k), c, in_f-within-chunk]
        tw = t_slice.reshape(IN_F // 128, 128, N_GRP, 128)
        tt = np.ascontiguousarray(tw.transpose(2, 1, 0, 3)).astype(BF16)
        P = P_all[:, core * O_PER_CORE:(core + 1) * O_PER_CORE]  # [B, 16]
        Pb = P.astype(BF16)                   # bf16-quantized, used as-is
        Pf = Pb.astype(np.float32)
        # c1[i, q, row(jj,c)] = -bf16(P_S); bq[row, q] = -f32(bf16(P_S))[j]
        c1 = np.zeros((B, N_QUAD, 128), dtype=BF16)
        bq = np.zeros((128, N_QUAD), dtype=np.float32)
        for q in range(N_QUAD):
            for jj in range(JQ):
                labels = _REG[q][jj]
                mask = np.array([labels[c // O_PER_GRP] != "A"
                                 for c in range(O_PER_CORE)])
                c1[:, q, MW * jj:MW * jj + O_PER_CORE] = \
                    np.where(mask[None, :], -Pf, 0.0).astype(BF16)
                bq[MW * jj:MW * jj + O_PER_CORE, q] = \
                    np.where(mask, -Pf[JQ * q + jj, :], 0.0)
        in_maps.append({"tt": tt, "xt": xt, "s2b": s2b, "s8": s8,
                        "sel4": sel4, "idm": ident, "c1": c1, "bq": bq})

    res = run_bass_kernel_spmd(nc, in_maps, core_ids=list(range(N_CORES)))

    ob_full = np.empty((B, OUT_F), dtype=np.float32)
    for c, r in enumerate(res.results):
        rs = r["rs"]                                            # [128, 32]
        ac = r["accs"]                                          # [16, 124]
        row = rs.reshape(JQ, MW, N_QUAD)[:, :O_PER_CORE, :]     # [jj, r, q]
        ob = row.transpose(2, 0, 1).reshape(B, O_PER_CORE)      # [j, r]
        ob[JQ:, :] += ac.T                                      # j >= 4
        ob_full[:, c * O_PER_CORE:(c + 1) * O_PER_CORE] = ob
    out = np.concatenate([x, ob_full - 1.0], axis=1).astype(np.float32)
    return out


if __name__ == "__main__":
    print("plan loads (ns):", {k: round(v) for k, v in _LOAD.items()})
    n8 = sum(p[0] + p[1] for p in _PAIR8)
    print(f"fp8 DR pairs: {n8}/64")


# revision 11
# speedup vs baseline: 1.0599x; 1.0308x over previous
"""Minibatch discrimination kernel for 8 Trainium2 NeuronCores.

Reference computation:
    m = (x @ T.reshape(512, 128*32)).reshape(B=128, O=128, K=32)
    norm[i,j,o] = sum_k |m[i,o,k] - m[j,o,k]|
    o_b[j,o]    = sum_i exp(-norm[i,j,o]) - 1
    out         = concat([x, o_b], axis=1)            # [128, 640]

Distribution: shard the output-feature dim O=128 across the 8 cores (16
o's per core); no collectives.  Each core runs the GEMM for its T-slice
and the BxB pairwise exp-sum for its o-slice.

Per-core dataflow (tiles are [partition, free]):
  - GEMM -> M per o-group g as [(4o x 32k)=128 partitions, i=128] in
    PSUM; evicted to bf16 m_bf plus an exact f32 upcast m32 (the
    per-partition scalar / activation bias source).
  - norm is symmetric, so quad q (4 j's) only computes columns
    i >= 4q: free dim shrinks 128 -> 4 across quads, halving the
    elementwise volume.  The missing i < 4q part of o_b comes back via
    per-quad column sums (see below).
  - |d| tiles, one fused op per (j-region, o-group):
      DVE / GpSimd: tensor_scalar(subtract, max, 0) = relu(m_i - m_j)
        (weight-2 selector + P-correction; abs is not in the DVE/Pool
        hw ISA),
      ScalarE: activation Abs(-m + bias m32[:,j]) = |d| directly
        (weight-1 selector, no correction).
    A static plan balances the three engines' busy time.
  - k-reduction on the TensorEngine: per quad one seed matmul deposits
    -P_S[i,o] (P_S = sum over the RELU-produced groups of that row's
    region, host-precomputed per quad in c1) and per tile a selector
    matmul accumulates into the [(4j x 32(16o+16pad)), i] PSUM tile.
    Region jj=0 packs its tiles as fp8 pairs consumed by DoubleRow
    matmuls (0.5 cyc/row; hw requires dst partition base 0, so only
    this region qualifies).
  - One ScalarE Exp per quad with bias +P_S[j,o] (rides the bq table)
    -> E_q in SBUF bf16.  Row sums via DVE free-axis reduce into
    rs[:, q]; column sums via one PE matmul per quad accumulating
    sel4^T @ E_q[:, 4:] into ACC[16 o, j] over columns j >= 4q+4 only,
    so ACC[o,j] ends as sum_{i<4q_j} E[i,j] (prefix by construction).
  - Host combines o_b[j,o] = rs + ACC - 1 and concats with x.
"""

import numpy as np
import ml_dtypes

import concourse.bacc as bacc
import concourse.tile as tile
import concourse.mybir as mybir
from concourse.bass_utils import run_bass_kernel_spmd

BF16 = ml_dtypes.bfloat16
FP8 = ml_dtypes.float8_e4m3

B = 128          # batch
IN_F = 512       # in_features
OUT_F = 128      # out_features
KD = 32          # kernel dim
N_CORES = 8
O_PER_CORE = OUT_F // N_CORES        # 16
N_GRP = O_PER_CORE * KD // 128       # 4 o-groups of (4 o x 32 k) partitions
O_PER_GRP = 128 // KD                # 4
JQ = 4                               # j's per PSUM tile / exp instruction
N_QUAD = B // JQ                     # 32
MW = 32                              # matmul M width per j (16 real + 16 zero)
LAG = 3                              # quads between exp and rowsum/colsum
ELAG = 2                             # quads between norm-psum and exp


def _plan():
    """Static engine plan for the 512 (q, jj, g) |d| tiles.

    Greedy makespan balance using the TimelineSim engine-busy costs
    (f = 128-4q): DVE 60.4+0.260f, ScalarE 185+0.833f, GpSimd
    95+1.389f.  Fixed loads: DVE carries evictions/upcasts/rowsums,
    ScalarE the exps.  Tiles within a quad are interchangeable, so the
    per-quad engine multiset is then packed into regions with the
    non-DVE tiles concentrated at region 0 (the only DoubleRow-legal
    dst), paired for fp8.

    Returns regions[q][jj] = list of 4 labels in {'D','A','P'} and
    pair8[q] = (pair0_is_fp8, pair1_is_fp8) for region 0.
    """
    load = {
        "D": 4 * 258 + 4 * 127 + 254 + sum(60.4 + 0.260 * (128 - 4 * q)
                                           for q in range(N_QUAD)),
        "A": sum(185 + 0.833 * (128 - 4 * q) for q in range(N_QUAD)),
        "P": 0.0,
    }
    cost = {
        "D": lambda f: 60.4 + 0.260 * f,
        "A": lambda f: 185 + 0.833 * f,
        "P": lambda f: 95 + 1.389 * f,
    }
    counts = [{"D": 0, "A": 0, "P": 0} for _ in range(N_QUAD)]
    tiles = [(128 - 4 * q, q) for q in range(N_QUAD)
             for _ in range(JQ * N_GRP)]
    tiles.sort(key=lambda t: -t[0])
    for f, q in tiles:
        pick = min(cost, key=lambda e: load[e] + cost[e](f))
        load[pick] += cost[pick](f)
        counts[q][pick] += 1
    regions = []
    pair8 = []
    for q in range(N_QUAD):
        c = dict(counts[q])
        nond = ["P"] * c["P"] + ["A"] * c["A"]
        dd = ["D"] * c["D"]
        r0 = [(nond or dd).pop(0) for _ in range(N_GRP)]
        rest = nond + dd
        regs = [r0] + [[rest.pop(0) for _ in range(N_GRP)]
                       for _ in range(1, JQ)]
        pair8.append((r0[0] != "D" and r0[1] != "D",
                      r0[2] != "D" and r0[3] != "D"))
        regions.append(regs)
    return regions, pair8, load


_REG, _PAIR8, _LOAD = _plan()


def _build():
    f32, bf16 = mybir.dt.float32, mybir.dt.bfloat16
    fp8 = mybir.dt.float8e4
    A = mybir.AluOpType
    AF = mybir.ActivationFunctionType
    nc = bacc.Bacc("TRN2", target_bir_lowering=False, debug=False)

    tt_d = nc.dram_tensor("tt", [N_GRP, 128, IN_F // 128, 128], bf16, kind="ExternalInput")
    xt_d = nc.dram_tensor("xt", [128, IN_F // 128, B], bf16, kind="ExternalInput")
    s2b_d = nc.dram_tensor("s2b", [128, 2, N_GRP, MW], bf16, kind="ExternalInput")
    s8_d = nc.dram_tensor("s8", [128, 2, 4, 2, MW], fp8, kind="ExternalInput")
    sel4_d = nc.dram_tensor("sel4", [128, O_PER_CORE], bf16, kind="ExternalInput")
    id_d = nc.dram_tensor("idm", [128, 128], bf16, kind="ExternalInput")
    c1_d = nc.dram_tensor("c1", [B, N_QUAD, 128], bf16, kind="ExternalInput")
    bq_d = nc.dram_tensor("bq", [128, N_QUAD], f32, kind="ExternalInput")
    rs_d = nc.dram_tensor("rs", [128, N_QUAD], f32, kind="ExternalOutput")
    acc_d = nc.dram_tensor("accs", [O_PER_CORE, B - JQ], f32, kind="ExternalOutput")

    n_chunk = IN_F // 128  # 4 contraction chunks

    with tile.TileContext(nc) as tc:
        with (
            tc.tile_pool(name="singles", bufs=1) as singles,
            tc.tile_pool(name="adpool", bufs=96) as adpool,
            tc.tile_pool(name="a8pool", bufs=32) as a8pool,
            tc.tile_pool(name="epool", bufs=LAG + 3) as epool,
            tc.tile_pool(name="psn", bufs=5, space="PSUM") as psn,
            tc.tile_pool(name="psa", bufs=1, space="PSUM") as psa,
        ):
            # --- warm the ACT exp/abs tables while DMAs run ---
            warm = singles.tile([1, 4], mybir.dt.float32, tag="warm")
            nc.vector.memset(warm[:], 0.0)
            nc.scalar.activation(
                out=warm[0:1, 0:1], in_=warm[0:1, 1:2],
                func=AF.Exp, bias=0.0, scale=-1.0,
            )
            nc.scalar.activation(
                out=warm[0:1, 2:3], in_=warm[0:1, 3:4],
                func=AF.Abs, bias=0.0, scale=-1.0,
            )

            # --- batched input DMAs, all on the SP queue ---
            # tt: dram [512, 512] -> sbuf [128, 4c, 512]
            t_sb = singles.tile([128, N_GRP, n_chunk, 128], bf16, tag="t")
            x_sb = singles.tile([128, n_chunk, B], bf16, tag="x")
            c1_sb = singles.tile([B, N_QUAD, 128], bf16, tag="c1")
            nc.sync.dma_start(t_sb[:, 0, :, :], tt_d[0].transpose([0, 1, 2]))
            nc.sync.dma_start(x_sb[:], xt_d[:])
            nc.sync.dma_start(t_sb[:, 1, :, :], tt_d[1])
            nc.sync.dma_start(t_sb[:, 2, :, :], tt_d[2])
            nc.sync.dma_start(t_sb[:, 3, :, :], tt_d[3])
            nc.sync.dma_start(c1_sb[:, 0:8, :], c1_d[:, 0:8, :])
            s2b_sb = singles.tile([128, 2, N_GRP, MW], bf16, tag="s2b")
            nc.sync.dma_start(s2b_sb[:], s2b_d[:])
            s8_sb = singles.tile([128, 2, 4, 2, MW], fp8, tag="s8")
            nc.sync.dma_start(s8_sb[:], s8_d[:])
            sel4_sb = singles.tile([128, O_PER_CORE], bf16, tag="sel4")
            nc.sync.dma_start(sel4_sb[:], sel4_d[:])
            id_sb = singles.tile([128, 128], bf16, tag="idm")
            nc.sync.dma_start(id_sb[:], id_d[:])
            bq_sb = singles.tile([128, N_QUAD], f32, tag="bq")
            nc.sync.dma_start(bq_sb[:], bq_d[:])
            nc.sync.dma_start(c1_sb[:, 8:20, :], c1_d[:, 8:20, :])
            nc.sync.dma_start(c1_sb[:, 20:32, :], c1_d[:, 20:32, :])

            # --- GEMM: M[g] = (T_g)^T x^T : [(4o,32k)=128, i=128] ---
            m_bf = []
            m32 = []
            gemm_pool_cm = tc.tile_pool(name="psg", bufs=2, space="PSUM")
            psg = gemm_pool_cm.__enter__()
            for g in range(N_GRP):
                pg = psg.tile([128, B], f32, tag="gemm")
                for c in range(n_chunk):
                    nc.tensor.matmul(
                        pg[:],
                        t_sb[:, g, c, :],
                        x_sb[:, c, :],
                        start=(c == 0),
                        stop=(c == n_chunk - 1),
                    )
                mb = singles.tile([128, B], bf16, tag=f"mb{g}")
                nc.vector.tensor_copy(mb[:], pg[:])   # PSUM -> SBUF, bf16
                m_bf.append(mb)
                mu = singles.tile([128, B], f32, tag=f"mu{g}")
                nc.vector.tensor_copy(mu[:], mb[:])   # exact f32 upcast
                m32.append(mu)
            gemm_pool_cm.__exit__(None, None, None)

            # --- pairwise: per j-quad, |d| tiles -> k-reduce -> exp ---
            rs_sb = singles.tile([128, N_QUAD], f32, tag="rs")
            acc_ps = psa.tile([O_PER_CORE, B], f32, tag="accp")
            pending = []

            def emit_tile(eng, dst, g, j, i0):
                if eng == "D":
                    nc.vector.tensor_scalar(
                        out=dst, in0=m_bf[g][:, i0:B],
                        scalar1=m32[g][:, j:j + 1], scalar2=0.0,
                        op0=A.subtract, op1=A.max,
                    )
                elif eng == "A":
                    nc.scalar.activation(
                        out=dst, in_=m_bf[g][:, i0:B],
                        func=AF.Abs,
                        bias=m32[g][:, j:j + 1], scale=-1.0,
                    )
                else:
                    nc.gpsimd.tensor_scalar(
                        out=dst, in0=m_bf[g][:, i0:B],
                        scalar1=m32[g][:, j:j + 1], scalar2=0.0,
                        op0=A.subtract, op1=A.max,
                    )

            exp_pending = []

            def emit_exp(q, pn_tile, f):
                e_tile = epool.tile([128, B], bf16, tag="e")
                nc.scalar.activation(
                    out=e_tile[:, 0:f], in_=pn_tile[:, 0:f],
                    func=AF.Exp, bias=bq_sb[:, q:q + 1], scale=-1.0,
                )
                pending.append((q, e_tile, f))
                if len(pending) > LAG:
                    finish(*pending.pop(0))

            def finish(q, e_tile, f):
                nc.vector.tensor_reduce(
                    out=rs_sb[:, q:q + 1], in_=e_tile[:, 0:f],
                    axis=mybir.AxisListType.X, op=A.add,
                )
                if q == 15:
                    nc.sync.dma_start(rs_d[:, 0:16], rs_sb[:, 0:16])
                elif q == 27:
                    nc.sync.dma_start(rs_d[:, 16:28], rs_sb[:, 16:28])
                if q < N_QUAD - 1:
                    nc.tensor.matmul(
                        acc_ps[:, 4 * q + JQ:B], sel4_sb[:],
                        e_tile[:, JQ:f],
                        start=(q == 0), stop=(q == N_QUAD - 2),
                        skip_group_check=True,
                    )

            for q in range(N_QUAD):
                i0 = 4 * q
                f = 128 - i0
                pn = psn.tile([128, B], f32, tag="norm")
                # seed: pn[row, i] = -P_S(row)[i, o(row)] for all 4 regions
                nc.tensor.matmul(
                    pn[:, 0:f], c1_sb[:, q, :], id_sb[:, i0:B],
                    start=True, stop=False, skip_group_check=True,
                )
                for jj in range(JQ):
                    j = JQ * q + jj
                    labels = _REG[q][jj]
                    reg = pn[MW * jj:MW * (jj + 1), 0:f]
                    mms = []
                    if jj == 0:
                        for pr in range(2):
                            gs = (2 * pr, 2 * pr + 1)
                            if _PAIR8[q][pr]:
                                a8 = a8pool.tile([128, 2, B], fp8, tag="a8")
                                for t, g in enumerate(gs):
                                    emit_tile(labels[g], a8[:, t, 0:f], g, j, i0)
                                # selector variant by (slot0, slot1) weights
                                v = ((labels[gs[0]] == "P") * 2
                                     + (labels[gs[1]] == "P") * 1)
                                mms.append(("dr", pr, v, a8))
                            else:
                                for g in gs:
                                    ad = adpool.tile([128, B], bf16, tag="ad")
                                    emit_tile(labels[g], ad[:, 0:f], g, j, i0)
                                    mms.append(("b", g, labels[g], ad))
                    else:
                        for g in range(N_GRP):
                            ad = adpool.tile([128, B], bf16, tag="ad")
                            emit_tile(labels[g], ad[:, 0:f], g, j, i0)
                            mms.append(("b", g, labels[g], ad))
                    for idx, mm in enumerate(mms):
                        stop = idx == len(mms) - 1
                        if mm[0] == "dr":
                            _, pr, v, a8 = mm
                            nc.tensor.matmul(
                                reg, s8_sb[:, pr, v, :, :], a8[:, :, 0:f],
                                start=False, stop=stop,
                                perf_mode=mybir.MatmulPerfMode.DoubleRow,
                                tile_position=(0, MW * jj),
                                skip_group_check=True,
                            )
                        else:
                            _, g, lab, ad = mm
                            w = 0 if lab != "A" else 1   # 0: weight 2, 1: weight 1
                            nc.tensor.matmul(
                                reg, s2b_sb[:, w, g, :], ad[:, 0:f],
                                start=False, stop=stop,
                                tile_position=(0, MW * jj),
                                skip_group_check=True,
                            )

                exp_pending.append((q, pn, f))
                if len(exp_pending) > ELAG:
                    emit_exp(*exp_pending.pop(0))

            while exp_pending:
                emit_exp(*exp_pending.pop(0))
            while pending:
                finish(*pending.pop(0))

            # --- ship results ---
            acc_sb = singles.tile([O_PER_CORE, B - JQ], f32, tag="acc_sb")
            nc.vector.tensor_copy(acc_sb[:], acc_ps[:, JQ:B])
            nc.sync.dma_start(rs_d[:, 28:N_QUAD], rs_sb[:, 28:N_QUAD])
            nc.sync.dma_start(acc_d[:], acc_sb[:])

    nc.compile()
    return nc


_NC = None


def kernel(x: np.ndarray, T: np.ndarray) -> np.ndarray:
    global _NC
    if _NC is None:
        _NC = _build()
    nc = _NC

    x = np.ascontiguousarray(x, dtype=np.float32)
    T = np.ascontiguousarray(T, dtype=np.float32)

    xt = np.ascontiguousarray(
        x.T.reshape(IN_F // 128, 128, B).transpose(1, 0, 2)).astype(BF16)

    # selectors: col g*4 + o_loc, weight 2 (relu tiles) or 1 (abs tiles)
    s2b = np.zeros((128, 2, N_GRP, MW), dtype=BF16)
    for p in range(128):
        o_loc = p // KD
        for g in range(N_GRP):
            s2b[p, 0, g, g * O_PER_GRP + o_loc] = 2
            s2b[p, 1, g, g * O_PER_GRP + o_loc] = 1
    # fp8 DoubleRow selector: pair pr covers groups (2pr, 2pr+1); variant
    # v encodes (slot0_weight==2)*2 + (slot1_weight==2)*1
    s8 = np.zeros((128, 2, 4, 2, MW), dtype=FP8)
    for p in range(128):
        o_loc = p // KD
        for pr in range(2):
            for v in range(4):
                w0 = 2 if v & 2 else 1
                w1 = 2 if v & 1 else 1
                g0, g1 = 2 * pr, 2 * pr + 1
                s8[p, pr, v, 0, g0 * O_PER_GRP + o_loc] = w0
                s8[p, pr, v, 1, g1 * O_PER_GRP + o_loc] = w1
    # colsum selector: partition (jj, c) -> column c (c < 16)
    sel4 = np.zeros((128, O_PER_CORE), dtype=BF16)
    for jj in range(JQ):
        for c in range(O_PER_CORE):
            sel4[MW * jj + c, c] = 1
    ident = np.eye(128, dtype=BF16)

    # host-side P[i, o] = sum_k m[i, o, k]; each o belongs to exactly one
    # group g = (o mod 16) // 4, so the relu correction for row (jj, o)
    # is P[i, o] masked by whether that region's group-g tile is
    # relu-produced (label != 'A').  Only consistency with the device's
    # bf16 m matters (the +P/-P copies cancel exactly on the diagonal).
    m_host = (x @ T.reshape(IN_F, OUT_F * KD)).reshape(B, OUT_F, KD)
    P_all = m_host.sum(axis=-1)                                 # [B, 128]

    in_maps = []
    for core in range(N_CORES):
        t_slice = T[:, core * O_PER_CORE:(core + 1) * O_PER_CORE, :]
        # [g][p=(4o,32k), c, in_f-within-chunk]
        tw = t_slice.reshape(IN_F // 128, 128, N_GRP, 128)
        tt = np.ascontiguousarray(tw.transpose(2, 1, 0, 3)).astype(BF16)
        P = P_all[:, core * O_PER_CORE:(core + 1) * O_PER_CORE]  # [B, 16]
        Pb = P.astype(BF16)                   # bf16-quantized, used as-is
        Pf = Pb.astype(np.float32)
        # c1[i, q, row(jj,c)] = -bf16(P_S); bq[row, q] = -f32(bf16(P_S))[j]
        c1 = np.zeros((B, N_QUAD, 128), dtype=BF16)
        bq = np.zeros((128, N_QUAD), dtype=np.float32)
        for q in range(N_QUAD):
            for jj in range(JQ):
                labels = _REG[q][jj]
                mask = np.array([labels[c // O_PER_GRP] != "A"
                                 for c in range(O_PER_CORE)])
                c1[:, q, MW * jj:MW * jj + O_PER_CORE] = \
                    np.where(mask[None, :], -Pf, 0.0).astype(BF16)
                bq[MW * jj:MW * jj + O_PER_CORE, q] = \
                    np.where(mask, -Pf[JQ * q + jj, :], 0.0)
        in_maps.append({"tt": tt, "xt": xt, "s2b": s2b, "s8": s8,
                        "sel4": sel4, "idm": ident, "c1": c1, "bq": bq})

    res = run_bass_kernel_spmd(nc, in_maps, core_ids=list(range(N_CORES)))

    ob_full = np.empty((B, OUT_F), dtype=np.float32)
    for c, r in enumerate(res.results):
        rs = r["rs"]                                            # [128, 32]
        ac = r["accs"]                                          # [16, 124]
        row = rs.reshape(JQ, MW, N_QUAD)[:, :O_PER_CORE, :]     # [jj, r, q]
        ob = row.transpose(2, 0, 1).reshape(B, O_PER_CORE)      # [j, r]
        ob[JQ:, :] += ac.T                                      # j >= 4
        ob_full[:, c * O_PER_CORE:(c + 1) * O_PER_CORE] = ob
    out = np.concatenate([x, ob_full - 1.0], axis=1).astype(np.float32)
    return out


if __name__ == "__main__":
    print("plan loads (ns):", {k: round(v) for k, v in _LOAD.items()})
    n8 = sum(p[0] + p[1] for p in _PAIR8)
    print(f"fp8 DR pairs: {n8}/64")


# revision 12
# speedup vs baseline: 1.3494x; 1.2731x over previous
"""Minibatch discrimination kernel for 8 Trainium2 NeuronCores.

Reference computation:
    m = (x @ T.reshape(512, 128*32)).reshape(B=128, O=128, K=32)
    norm[i,j,o] = sum_k |m[i,o,k] - m[j,o,k]|
    o_b[j,o]    = sum_i exp(-norm[i,j,o]) - 1
    out         = concat([x, o_b], axis=1)            # [128, 640]

Distribution: shard the output-feature dim O=128 across the 8 cores (16
o's per core); no collectives.  Each core runs the GEMM for its T-slice
and the BxB pairwise exp-sum for its o-slice.

Per-core dataflow (tiles are [partition, free]):
  - GEMM -> M per o-group g as [(4o x 32k)=128 partitions, i=128] in
    PSUM; evicted to bf16 m_bf plus an exact f32 upcast m32 (the
    per-partition scalar / activation bias source).
  - norm is symmetric, so quad q (4 j's) only computes columns
    i >= 4q: free dim shrinks 128 -> 4 across quads, halving the
    elementwise volume.  The missing i < 4q part of o_b comes back via
    per-quad column sums (see below).
  - |d| tiles, one fused op per (j-region, o-group):
      DVE / GpSimd: tensor_scalar(subtract, max, 0) = relu(m_i - m_j)
        (weight-2 selector + P-correction; abs is not in the DVE/Pool
        hw ISA),
      ScalarE: activation Abs(-m + bias m32[:,j]) = |d| directly
        (weight-1 selector, no correction).
    A static plan balances the three engines' busy time.
  - k-reduction on the TensorEngine: per quad one seed matmul deposits
    -P_S[i,o] (P_S = sum over the RELU-produced groups of that row's
    region, host-precomputed per quad in c1) and per tile a selector
    matmul accumulates into the [(4j x 32(16o+16pad)), i] PSUM tile.
    Region jj=0 packs its tiles as fp8 pairs consumed by DoubleRow
    matmuls (0.5 cyc/row; hw requires dst partition base 0, so only
    this region qualifies).
  - One ScalarE Exp per quad with bias +P_S[j,o] (rides the bq table)
    -> E_q in SBUF bf16.  Row sums via DVE free-axis reduce into
    rs[:, q]; column sums via one PE matmul per quad accumulating
    sel4^T @ E_q[:, 4:] into ACC[16 o, j] over columns j >= 4q+4 only,
    so ACC[o,j] ends as sum_{i<4q_j} E[i,j] (prefix by construction).
  - Host combines o_b[j,o] = rs + ACC - 1 and concats with x.
"""

import numpy as np
import ml_dtypes

import concourse.bacc as bacc
import concourse.tile as tile
import concourse.mybir as mybir
from concourse.bass_utils import run_bass_kernel_spmd

BF16 = ml_dtypes.bfloat16
FP8 = ml_dtypes.float8_e4m3

B = 128          # batch
IN_F = 512       # in_features
OUT_F = 128      # out_features
KD = 32          # kernel dim
N_CORES = 8
O_PER_CORE = OUT_F // N_CORES        # 16
N_GRP = O_PER_CORE * KD // 128       # 4 o-groups of (4 o x 32 k) partitions
O_PER_GRP = 128 // KD                # 4
JQ = 4                               # j's per PSUM tile / exp instruction
N_QUAD = B // JQ                     # 32
MW = 32                              # matmul M width per j (16 real + 16 zero)
LAG = 3                              # quads between exp and rowsum/colsum
ELAG = 2                             # quads between norm-psum and exp


def _plan():
    """Static engine plan for the 512 (q, jj, g) |d| tiles.

    Greedy makespan balance using the TimelineSim engine-busy costs
    (f = 128-4q): DVE 60.4+0.260f, ScalarE 185+0.833f, GpSimd
    95+1.389f.  Fixed loads: DVE carries evictions/upcasts/rowsums,
    ScalarE the exps.  Tiles within a quad are interchangeable, so the
    per-quad engine multiset is then packed into regions with the
    non-DVE tiles concentrated at region 0 (the only DoubleRow-legal
    dst), paired for fp8.

    Returns regions[q][jj] = list of 4 labels in {'D','A','P'} and
    pair8[q] = (pair0_is_fp8, pair1_is_fp8) for region 0.
    """
    load = {
        "D": 4 * 258 + 4 * 127 + 254 + sum(60.4 + 0.260 * (128 - 4 * q)
                                           for q in range(N_QUAD)),
        "A": sum(185 + 0.833 * (128 - 4 * q) for q in range(N_QUAD)),
        "P": 0.0,
    }
    cost = {
        "D": lambda f: 60.4 + 0.260 * f,
        "A": lambda f: 185 + 0.833 * f,
        "P": lambda f: 95 + 1.389 * f,
    }
    counts = [{"D": 0, "A": 0, "P": 0} for _ in range(N_QUAD)]
    tiles = [(128 - 4 * q, q) for q in range(N_QUAD)
             for _ in range(JQ * N_GRP)]
    tiles.sort(key=lambda t: -t[0])
    for f, q in tiles:
        pick = min(cost, key=lambda e: load[e] + cost[e](f))
        load[pick] += cost[pick](f)
        counts[q][pick] += 1
    regions = []
    pair8 = []
    for q in range(N_QUAD):
        c = dict(counts[q])
        nond = ["P"] * c["P"] + ["A"] * c["A"]
        dd = ["D"] * c["D"]
        r0 = [(nond or dd).pop(0) for _ in range(N_GRP)]
        rest = nond + dd
        regs = [r0] + [[rest.pop(0) for _ in range(N_GRP)]
                       for _ in range(1, JQ)]
        pair8.append((r0[0] != "D" and r0[1] != "D",
                      r0[2] != "D" and r0[3] != "D"))
        regions.append(regs)
    return regions, pair8, load


_REG, _PAIR8, _LOAD = _plan()


def _build():
    f32, bf16 = mybir.dt.float32, mybir.dt.bfloat16
    fp8 = mybir.dt.float8e4
    A = mybir.AluOpType
    AF = mybir.ActivationFunctionType
    nc = bacc.Bacc("TRN2", target_bir_lowering=False, debug=False)

    tt_d = nc.dram_tensor("tt", [N_GRP, 128, IN_F // 128, 128], bf16, kind="ExternalInput")
    xt_d = nc.dram_tensor("xt", [128, IN_F // 128, B], bf16, kind="ExternalInput")
    s2b_d = nc.dram_tensor("s2b", [128, 2, N_GRP, MW], bf16, kind="ExternalInput")
    s8_d = nc.dram_tensor("s8", [128, 2, 4, 2, MW], fp8, kind="ExternalInput")
    sel4_d = nc.dram_tensor("sel4", [128, O_PER_CORE], bf16, kind="ExternalInput")
    id_d = nc.dram_tensor("idm", [128, 128], bf16, kind="ExternalInput")
    c1_d = nc.dram_tensor("c1", [B, N_QUAD, 128], bf16, kind="ExternalInput")
    bq_d = nc.dram_tensor("bq", [128, N_QUAD], f32, kind="ExternalInput")
    rs_d = nc.dram_tensor("rs", [128, N_QUAD], f32, kind="ExternalOutput")
    acc_d = nc.dram_tensor("accs", [O_PER_CORE, B - JQ], f32, kind="ExternalOutput")

    n_chunk = IN_F // 128  # 4 contraction chunks

    with tile.TileContext(nc) as tc:
        with (
            tc.tile_pool(name="singles", bufs=1) as singles,
            tc.tile_pool(name="psn", bufs=5, space="PSUM") as psn,
            tc.tile_pool(name="psa", bufs=1, space="PSUM") as psa,
        ):
            # --- warm the ACT exp/abs tables while DMAs run ---
            warm = singles.tile([1, 4], mybir.dt.float32, tag="warm")
            nc.vector.memset(warm[:], 0.0)
            nc.scalar.activation(
                out=warm[0:1, 0:1], in_=warm[0:1, 1:2],
                func=AF.Exp, bias=0.0, scale=-1.0,
            )
            nc.scalar.activation(
                out=warm[0:1, 2:3], in_=warm[0:1, 3:4],
                func=AF.Abs, bias=0.0, scale=-1.0,
            )

            # --- batched input DMAs, all on the SP queue ---
            # tt: dram [512, 512] -> sbuf [128, 4c, 512]
            t_sb = singles.tile([128, N_GRP, n_chunk, 128], bf16, tag="t")
            x_sb = singles.tile([128, n_chunk, B], bf16, tag="x")
            c1_sb = singles.tile([B, N_QUAD, 128], bf16, tag="c1")
            nc.sync.dma_start(t_sb[:, 0, :, :], tt_d[0].transpose([0, 1, 2]))
            nc.sync.dma_start(x_sb[:], xt_d[:])
            nc.sync.dma_start(t_sb[:, 1, :, :], tt_d[1])
            nc.sync.dma_start(t_sb[:, 2, :, :], tt_d[2])
            nc.sync.dma_start(t_sb[:, 3, :, :], tt_d[3])
            nc.sync.dma_start(c1_sb[:, 0:8, :], c1_d[:, 0:8, :])
            s2b_sb = singles.tile([128, 2, N_GRP, MW], bf16, tag="s2b")
            nc.sync.dma_start(s2b_sb[:], s2b_d[:])
            s8_sb = singles.tile([128, 2, 4, 2, MW], fp8, tag="s8")
            nc.sync.dma_start(s8_sb[:], s8_d[:])
            sel4_sb = singles.tile([128, O_PER_CORE], bf16, tag="sel4")
            nc.sync.dma_start(sel4_sb[:], sel4_d[:])
            id_sb = singles.tile([128, 128], bf16, tag="idm")
            nc.sync.dma_start(id_sb[:], id_d[:])
            bq_sb = singles.tile([128, N_QUAD], f32, tag="bq")
            nc.sync.dma_start(bq_sb[:], bq_d[:])
            nc.sync.dma_start(c1_sb[:, 8:20, :], c1_d[:, 8:20, :])
            nc.sync.dma_start(c1_sb[:, 20:32, :], c1_d[:, 20:32, :])

            # --- GEMM: M[g] = (T_g)^T x^T : [(4o,32k)=128, i=128] ---
            m_bf = []
            m32 = []
            gemm_pool_cm = tc.tile_pool(name="psg", bufs=2, space="PSUM")
            psg = gemm_pool_cm.__enter__()
            for g in range(N_GRP):
                pg = psg.tile([128, B], f32, tag="gemm")
                for c in range(n_chunk):
                    nc.tensor.matmul(
                        pg[:],
                        t_sb[:, g, c, :],
                        x_sb[:, c, :],
                        start=(c == 0),
                        stop=(c == n_chunk - 1),
                    )
                mb = singles.tile([128, B], bf16, tag=f"mb{g}")
                nc.vector.tensor_copy(mb[:], pg[:])   # PSUM -> SBUF, bf16
                m_bf.append(mb)
                mu = singles.tile([128, B], f32, tag=f"mu{g}")
                nc.vector.tensor_copy(mu[:], mb[:])   # exact f32 upcast
                m32.append(mu)
            gemm_pool_cm.__exit__(None, None, None)

            # --- pairwise: per j-quad, |d| tiles -> k-reduce -> exp ---
            rs_sb = singles.tile([128, N_QUAD], f32, tag="rs")
            acc_ps = psa.tile([O_PER_CORE, B], f32, tag="accp")
            pending = []

            def emit_tile(eng, dst, g, j, i0):
                if eng == "D":
                    nc.vector.tensor_scalar(
                        out=dst, in0=m_bf[g][:, i0:B],
                        scalar1=m32[g][:, j:j + 1], scalar2=0.0,
                        op0=A.subtract, op1=A.max,
                    )
                elif eng == "A":
                    nc.scalar.activation(
                        out=dst, in_=m_bf[g][:, i0:B],
                        func=AF.Abs,
                        bias=m32[g][:, j:j + 1], scale=-1.0,
                    )
                else:
                    nc.gpsimd.tensor_scalar(
                        out=dst, in0=m_bf[g][:, i0:B],
                        scalar1=m32[g][:, j:j + 1], scalar2=0.0,
                        op0=A.subtract, op1=A.max,
                    )

            exp_pending = []

            def emit_exp(q, pn_tile, f):
                e_tile = singles.tile([128, f], bf16, tag=f"e_{q}")
                nc.scalar.activation(
                    out=e_tile[:, 0:f], in_=pn_tile[:, 0:f],
                    func=AF.Exp, bias=bq_sb[:, q:q + 1], scale=-1.0,
                )
                pending.append((q, e_tile, f))
                if len(pending) > LAG:
                    finish(*pending.pop(0))

            def finish(q, e_tile, f):
                nc.vector.tensor_reduce(
                    out=rs_sb[:, q:q + 1], in_=e_tile[:, 0:f],
                    axis=mybir.AxisListType.X, op=A.add,
                )
                if q == 15:
                    nc.sync.dma_start(rs_d[:, 0:16], rs_sb[:, 0:16])
                elif q == 27:
                    nc.sync.dma_start(rs_d[:, 16:28], rs_sb[:, 16:28])
                if q < N_QUAD - 1:
                    nc.tensor.matmul(
                        acc_ps[:, 4 * q + JQ:B], sel4_sb[:],
                        e_tile[:, JQ:f],
                        start=(q == 0), stop=(q == N_QUAD - 2),
                        skip_group_check=True,
                    )

            for q in range(N_QUAD):
                i0 = 4 * q
                f = 128 - i0
                pn = psn.tile([128, B], f32, tag="norm")
                # seed: pn[row, i] = -P_S(row)[i, o(row)] for all 4 regions
                nc.tensor.matmul(
                    pn[:, 0:f], c1_sb[:, q, :], id_sb[:, i0:B],
                    start=True, stop=False, skip_group_check=True,
                )
                for jj in range(JQ):
                    j = JQ * q + jj
                    labels = _REG[q][jj]
                    reg = pn[MW * jj:MW * (jj + 1), 0:f]
                    mms = []
                    if jj == 0:
                        for pr in range(2):
                            gs = (2 * pr, 2 * pr + 1)
                            if _PAIR8[q][pr]:
                                f16 = (f + 15) // 16 * 16
                                a8 = singles.tile([128, 2, f16], fp8,
                                                  tag=f"a8_{q}_{pr}")
                                for t, g in enumerate(gs):
                                    emit_tile(labels[g], a8[:, t, 0:f], g, j, i0)
                                # selector variant by (slot0, slot1) weights
                                v = ((labels[gs[0]] == "P") * 2
                                     + (labels[gs[1]] == "P") * 1)
                                mms.append(("dr", pr, v, a8))
                            else:
                                for g in gs:
                                    ad = singles.tile([128, f], bf16,
                                                      tag=f"a_{q}_{jj}_{g}")
                                    emit_tile(labels[g], ad[:, 0:f], g, j, i0)
                                    mms.append(("b", g, labels[g], ad))
                    else:
                        for g in range(N_GRP):
                            ad = singles.tile([128, f], bf16,
                                              tag=f"a_{q}_{jj}_{g}")
                            emit_tile(labels[g], ad[:, 0:f], g, j, i0)
                            mms.append(("b", g, labels[g], ad))
                    for idx, mm in enumerate(mms):
                        stop = idx == len(mms) - 1
                        if mm[0] == "dr":
                            _, pr, v, a8 = mm
                            nc.tensor.matmul(
                                reg, s8_sb[:, pr, v, :, :], a8[:, :, 0:f],
                                start=False, stop=stop,
                                perf_mode=mybir.MatmulPerfMode.DoubleRow,
                                tile_position=(0, MW * jj),
                                skip_group_check=True,
                            )
                        else:
                            _, g, lab, ad = mm
                            w = 0 if lab != "A" else 1   # 0: weight 2, 1: weight 1
                            nc.tensor.matmul(
                                reg, s2b_sb[:, w, g, :], ad[:, 0:f],
                                start=False, stop=stop,
                                tile_position=(0, MW * jj),
                                skip_group_check=True,
                            )

                exp_pending.append((q, pn, f))
                if len(exp_pending) > ELAG:
                    emit_exp(*exp_pending.pop(0))

            while exp_pending:
                emit_exp(*exp_pending.pop(0))
            while pending:
                finish(*pending.pop(0))

            # --- ship results ---
            acc_sb = singles.tile([O_PER_CORE, B - JQ], f32, tag="acc_sb")
            nc.vector.tensor_copy(acc_sb[:], acc_ps[:, JQ:B])
            nc.sync.dma_start(rs_d[:, 28:N_QUAD], rs_sb[:, 28:N_QUAD])
            nc.sync.dma_start(acc_d[:], acc_sb[:])

    nc.compile()
    return nc


_NC = None


def kernel(x: np.ndarray, T: np.ndarray) -> np.ndarray:
    global _NC
    if _NC is None:
        _NC = _build()
    nc = _NC

    x = np.ascontiguousarray(x, dtype=np.float32)
    T = np.ascontiguousarray(T, dtype=np.float32)

    xt = np.ascontiguousarray(
        x.T.reshape(IN_F // 128, 128, B).transpose(1, 0, 2)).astype(BF16)

    # selectors: col g*4 + o_loc, weight 2 (relu tiles) or 1 (abs tiles)
    s2b = np.zeros((128, 2, N_GRP, MW), dtype=BF16)
    for p in range(128):
        o_loc = p // KD
        for g in range(N_GRP):
            s2b[p, 0, g, g * O_PER_GRP + o_loc] = 2
            s2b[p, 1, g, g * O_PER_GRP + o_loc] = 1
    # fp8 DoubleRow selector: pair pr covers groups (2pr, 2pr+1); variant
    # v encodes (slot0_weight==2)*2 + (slot1_weight==2)*1
    s8 = np.zeros((128, 2, 4, 2, MW), dtype=FP8)
    for p in range(128):
        o_loc = p // KD
        for pr in range(2):
            for v in range(4):
                w0 = 2 if v & 2 else 1
                w1 = 2 if v & 1 else 1
                g0, g1 = 2 * pr, 2 * pr + 1
                s8[p, pr, v, 0, g0 * O_PER_GRP + o_loc] = w0
                s8[p, pr, v, 1, g1 * O_PER_GRP + o_loc] = w1
    # colsum selector: partition (jj, c) -> column c (c < 16)
    sel4 = np.zeros((128, O_PER_CORE), dtype=BF16)
    for jj in range(JQ):
        for c in range(O_PER_CORE):
            sel4[MW * jj + c, c] = 1
    ident = np.eye(128, dtype=BF16)

    # host-side P[i, o] = sum_k m[i, o, k]; each o belongs to exactly one
    # group g = (o mod 16) // 4, so the relu correction for row (jj, o)
    # is P[i, o] masked by whether that region's group-g tile is
    # relu-produced (label != 'A').  Only consistency with the device's
    # bf16 m matters (the +P/-P copies cancel exactly on the diagonal).
    m_host = (x @ T.reshape(IN_F, OUT_F * KD)).reshape(B, OUT_F, KD)
    P_all = m_host.sum(axis=-1)                                 # [B, 128]

    in_maps = []
    for core in range(N_CORES):
        t_slice = T[:, core * O_PER_CORE:(core + 1) * O_PER_CORE, :]
        # [g][p=(4o,32k), c, in_f-within-chunk]
        tw = t_slice.reshape(IN_F // 128, 128, N_GRP, 128)
        tt = np.ascontiguousarray(tw.transpose(2, 1, 0, 3)).astype(BF16)
        P = P_all[:, core * O_PER_CORE:(core + 1) * O_PER_CORE]  # [B, 16]
        Pb = P.astype(BF16)                   # bf16-quantized, used as-is
        Pf = Pb.astype(np.float32)
        # c1[i, q, row(jj,c)] = -bf16(P_S); bq[row, q] = -f32(bf16(P_S))[j]
        c1 = np.zeros((B, N_QUAD, 128), dtype=BF16)
        bq = np.zeros((128, N_QUAD), dtype=np.float32)
        for q in range(N_QUAD):
            for jj in range(JQ):
                labels = _REG[q][jj]
                mask = np.array([labels[c // O_PER_GRP] != "A"
                                 for c in range(O_PER_CORE)])
                c1[:, q, MW * jj:MW * jj + O_PER_CORE] = \
                    np.where(mask[None, :], -Pf, 0.0).astype(BF16)
                bq[MW * jj:MW * jj + O_PER_CORE, q] = \
                    np.where(mask, -Pf[JQ * q + jj, :], 0.0)
        in_maps.append({"tt": tt, "xt": xt, "s2b": s2b, "s8": s8,
                        "sel4": sel4, "idm": ident, "c1": c1, "bq": bq})

    res = run_bass_kernel_spmd(nc, in_maps, core_ids=list(range(N_CORES)))

    ob_full = np.empty((B, OUT_F), dtype=np.float32)
    for c, r in enumerate(res.results):
        rs = r["rs"]                                            # [128, 32]
        ac = r["accs"]                                          # [16, 124]
        row = rs.reshape(JQ, MW, N_QUAD)[:, :O_PER_CORE, :]     # [jj, r, q]
        ob = row.transpose(2, 0, 1).reshape(B, O_PER_CORE)      # [j, r]
        ob[JQ:, :] += ac.T                                      # j >= 4
        ob_full[:, c * O_PER_CORE:(c + 1) * O_PER_CORE] = ob
    out = np.concatenate([x, ob_full - 1.0], axis=1).astype(np.float32)
    return out


if __name__ == "__main__":
    print("plan loads (ns):", {k: round(v) for k, v in _LOAD.items()})
    n8 = sum(p[0] + p[1] for p in _PAIR8)
    print(f"fp8 DR pairs: {n8}/64")
